# revision 1
# baseline (speedup 1.0000x reference)
"""Grouped SwiGLU MoE FFN (8 experts) on 8 Trainium2 NeuronCores.

Expert-parallel: core e owns expert e's weights and its contiguous slice of
tokens (inputs arrive pre-sorted by expert).  Per core we compute
    g = silu(x_e @ w1_e.T); u = x_e @ w3_e.T; y_e = (g*u) @ w2_e.T
with fp32r (1-pass FP22) matmuls on the PE array.

All matmul operands need the contraction dim on SBUF partitions, so the host
pre-packs x/w1/w3/w2 into partition-major tiled layouts (pure numpy
transposes) and un-packs the output.  Device kernel processes tokens in two
halves of 1024 to fit SBUF.
"""

import sys

sys.path.insert(0, "/opt/trn_rl_repo")

import numpy as np

import concourse.bass as bass
import concourse.mybir as mybir
import concourse.tile as tile
from concourse import bacc
from concourse.bass import ts
from concourse.bass_utils import run_bass_kernel_spmd

F32 = mybir.dt.float32
F32R = mybir.dt.float32r

E, H, D, T = 8, 1408, 2048, 16384
TE = T // E            # tokens per expert (uniform fast path)
TH = TE // 2           # half of tokens processed at a time
NT = TH // 512         # 512-wide t tiles per half
KD = D // 128          # contraction tiles over d
JH = H // 128          # contraction tiles over h / h strips
ID = D // 128          # output d strips


def _build_program():
    nc = bacc.Bacc("TRN2", target_bir_lowering=False, debug=False, num_devices=E)

    xt_d = nc.dram_tensor("xt", [2, 128, KD, TH], F32, kind="ExternalInput").ap()
    w13_d = nc.dram_tensor("w13", [JH, 2, 128, KD, 128], F32, kind="ExternalInput").ap()
    w2_d = nc.dram_tensor("w2t", [ID, 128, JH, 128], F32, kind="ExternalInput").ap()
    y_d = nc.dram_tensor("y", [2, ID, 128, TH], F32, kind="ExternalOutput").ap()

    with tile.TileContext(nc) as tc:
        with (
            tc.tile_pool(name="xp", bufs=1) as xp,
            tc.tile_pool(name="wp", bufs=3) as wp,
            tc.tile_pool(name="hp", bufs=1) as hp,
            tc.tile_pool(name="sp", bufs=2) as sp,
            tc.tile_pool(name="yp", bufs=2) as yp,
            tc.tile_pool(name="ps", bufs=2, space="PSUM") as ps,
        ):
            for hf in range(2):
                # First matmul needs only w13[j=0, s=0] and xt[k=0, t<512];
                # issue DMAs in exactly the order the j=0 matmul stream
                # consumes them so PE starts as early as possible.
                w13_next = wp.tile([128, 2, KD, 128], F32R, tag="w13", name="w13p")
                nc.sync.dma_start(w13_next[:, 0], w13_d[0, 0].bitcast(F32R))
                xt = xp.tile([128, KD, TH], F32R, tag="xt")
                for tt in range(NT):
                    nc.sync.dma_start(
                        xt[:, 0, ts(tt, 512)], xt_d[hf, :, 0, ts(tt, 512)].bitcast(F32R)
                    )
                nc.sync.dma_start(w13_next[:, 1], w13_d[0, 1].bitcast(F32R))
                for k in range(1, KD):
                    for tt in range(NT):
                        nc.sync.dma_start(
                            xt[:, k, ts(tt, 512)],
                            xt_d[hf, :, k, ts(tt, 512)].bitcast(F32R),
                        )

                hh = []
                for j in range(JH):
                    w13 = w13_next
                    if j + 1 < JH:
                        w13_next = wp.tile(
                            [128, 2, KD, 128], F32R, tag="w13", name="w13p"
                        )
                        nc.sync.dma_start(w13_next[:, 0], w13_d[j + 1, 0].bitcast(F32R))
                        nc.sync.dma_start(w13_next[:, 1], w13_d[j + 1, 1].bitcast(F32R))

                    hh_j = hp.tile([128, TH], F32R, tag=f"hh{j}")
                    pg = [ps.tile([128, 512], F32, tag="pg", name=f"pg{tt}") for tt in range(NT)]
                    pu = [ps.tile([128, 512], F32, tag="pu", name=f"pu{tt}") for tt in range(NT)]
                    for k in range(KD):
                        for tt in range(NT):
                            nc.tensor.matmul(
                                pg[tt][:], w13[:, 0, k, :], xt[:, k, ts(tt, 512)],
                                start=(k == 0), stop=(k == KD - 1),
                            )
                    for k in range(KD):
                        for tt in range(NT):
                            nc.tensor.matmul(
                                pu[tt][:], w13[:, 1, k, :], xt[:, k, ts(tt, 512)],
                                start=(k == 0), stop=(k == KD - 1),
                            )
                    for tt in range(NT):
                        sg = sp.tile([128, 512], F32, tag="sg")
                        nc.scalar.activation(
                            sg[:], pg[tt][:], mybir.ActivationFunctionType.Silu
                        )
                        nc.vector.tensor_mul(hh_j[:, ts(tt, 512)], sg[:], pu[tt][:])
                    hh.append(hh_j)

                for i in range(ID):
                    w2 = wp.tile([128, JH, 128], F32R, tag="w2")
                    nc.sync.dma_start(w2[:], w2_d[i].bitcast(F32R))
                    y_sb = yp.tile([128, TH], F32, tag="ysb")
                    for tt in range(NT):
                        py = ps.tile([128, 512], F32, tag="py")
                        for j in range(JH):
                            nc.tensor.matmul(
                                py[:], w2[:, j, :], hh[j][:, ts(tt, 512)],
                                start=(j == 0), stop=(j == JH - 1),
                            )
                        nc.vector.tensor_copy(y_sb[:, ts(tt, 512)], py[:])
                        nc.sync.dma_start(
                            y_d[hf, i, :, ts(tt, 512)], y_sb[:, ts(tt, 512)]
                        )

    nc.compile()
    return nc


_NC = None


def _get_nc():
    global _NC
    if _NC is None:
        _NC = _build_program()
    return _NC


def _prep_core_inputs(x_e, w1_e, w3_e, w2_e):
    # xt[hf, p, k, t] = x_e[hf*TH + t, k*128 + p]
    xt = np.empty((2, 128, KD, TH), dtype=np.float32)
    for hf in range(2):
        xh = x_e[hf * TH:(hf + 1) * TH].T           # [D, TH]
        xt[hf] = xh.reshape(KD, 128, TH).transpose(1, 0, 2)
    # w13[j, s, p, k, h] = w{1,3}_e[j*128 + h, k*128 + p]
    w1r = w1_e.reshape(JH, 128, KD, 128).transpose(0, 3, 2, 1)
    w3r = w3_e.reshape(JH, 128, KD, 128).transpose(0, 3, 2, 1)
    w13 = np.ascontiguousarray(np.stack([w1r, w3r], axis=1))
    # w2t[i, p, j, dd] = w2_e[i*128 + dd, j*128 + p]
    w2t = np.ascontiguousarray(w2_e.reshape(ID, 128, JH, 128).transpose(0, 3, 2, 1))
    return {
        "xt": np.ascontiguousarray(xt),
        "w13": w13,
        "w2t": w2t,
    }


def _reference_fallback(w1, w2, w3, x, counts):
    # Exact numpy mirror of the jax reference (incl. scatter-drop / gather-clamp)
    e, h, d = w1.shape
    t = x.shape[0]
    cap = 2 * (t // e)
    counts = counts.astype(np.int64)
    offsets = np.concatenate([[0], np.cumsum(counts)[:-1]])
    eid = np.repeat(np.arange(e), counts)[:t]
    pos = np.arange(t) - offsets[eid]
    buf = np.zeros((e, cap, d), np.float32)
    ok = pos < cap
    buf[eid[ok], pos[ok]] = x[ok]
    out = np.empty((e, cap, d), np.float32)
    for ee in range(e):
        a = buf[ee] @ w1[ee].T
        g = a / (1.0 + np.exp(-a))
        u = buf[ee] @ w3[ee].T
        out[ee] = (g * u) @ w2[ee].T
    pos_c = np.minimum(pos, cap - 1)
    return out[eid, pos_c]


def kernel(w1, w2, w3, x, num_tokens_per_expert):
    w1 = np.asarray(w1, dtype=np.float32)
    w2 = np.asarray(w2, dtype=np.float32)
    w3 = np.asarray(w3, dtype=np.float32)
    x = np.asarray(x, dtype=np.float32)
    counts = np.asarray(num_tokens_per_expert).astype(np.int32)

    if not (x.shape == (T, D) and w1.shape == (E, H, D)
            and np.all(counts == TE)):
        return _reference_fallback(w1, w2, w3, x, counts)

    nc = _get_nc()
    in_maps = []
    for e in range(E):
        in_maps.append(
            _prep_core_inputs(x[e * TE:(e + 1) * TE], w1[e], w3[e], w2[e])
        )
    res = run_bass_kernel_spmd(nc, in_maps, list(range(E)))

    out = np.empty((T, D), dtype=np.float32)
    for e in range(E):
        y = res.results[e]["y"]  # [2, ID, 128, TH]
        for hf in range(2):
            out[e * TE + hf * TH: e * TE + (hf + 1) * TH] = (
                y[hf].reshape(D, TH).T
            )
    return out



# revision 6
# speedup vs baseline: 1.1741x; 1.1741x over previous
"""Grouped SwiGLU MoE FFN (8 experts) on 8 Trainium2 NeuronCores.

Expert-parallel: core e owns expert e's weights and its contiguous slice of
tokens (inputs arrive pre-sorted by expert).  Per core we compute
    g = silu(x_e @ w1_e.T); u = x_e @ w3_e.T; y_e = (g*u) @ w2_e.T

All matmuls run as fp8(e4m3) DoubleRow pairs (K=256 per instruction, 0.5
cycles/row) with hi/lo error compensation: every operand A is split into
A_hi = fp8(A) and A_lo = fp8(A - A_hi), and each product uses three terms
    A@B ~= A_hi@B_hi + A_hi@B_lo + A_lo@B_hi
which restores ~11-bit mantissa accuracy at 3/4 of the fp32r PE cost.
Weights are pre-scaled by 16 so their residuals stay out of the fp8
subnormal range; the silu input and the final output are descaled on the
scalar engine (activation scale).

Host pre-packs x/w1/w3/w2 into partition-major fp8 hi/lo tiles (numpy) and
un-packs the fp32 output.  The gu = silu(a)*u intermediate is quantized to
fp8 hi/lo pairs on-chip (ACT: silu + hi-quantize, DVE: mul + residual).
"""

import sys

sys.path.insert(0, "/opt/trn_rl_repo")

import numpy as np
import ml_dtypes

import concourse.bass as bass
import concourse.mybir as mybir
import concourse.tile as tile
from concourse import bacc
from concourse.bass import ts
from concourse.bass_utils import run_bass_kernel_spmd

F32 = mybir.dt.float32
F8 = mybir.dt.float8e4
DR = mybir.MatmulPerfMode.DoubleRow
NP_F8 = ml_dtypes.float8_e4m3

E, H, D, T = 8, 1408, 2048, 16384
TE = T // E            # tokens per expert (uniform fast path)
KD = D // 128          # contraction tiles over d (16)
KP = KD // 2           # DoubleRow k-pairs over d (8)
JH = H // 128          # h strips (11)
JHP = JH + 1           # h strips padded to even (12)
JP = JHP // 2          # DoubleRow jj-pairs over h (6)
ID = D // 128          # output d strips (16)
NT = TE // 512         # 512-token tiles (4)
SW = 16.0              # weight pre-scale
SG = 4.0               # gu pre-scale (fp8 overflow headroom)


def _build_program():
    nc = bacc.Bacc("TRN2", target_bir_lowering=False, debug=False, num_devices=E)

    xhi_d = nc.dram_tensor("xhi", [128, KD, TE], F8, kind="ExternalInput").ap()
    xlo_d = nc.dram_tensor("xlo", [128, KD, TE], F8, kind="ExternalInput").ap()
    w13hi_d = nc.dram_tensor("w13hi", [JH, 2, 128, KD, 128], F8,
                             kind="ExternalInput").ap()
    w13lo_d = nc.dram_tensor("w13lo", [JH, 2, 128, KD, 128], F8,
                             kind="ExternalInput").ap()
    w2hi_d = nc.dram_tensor("w2hi", [ID, 128, JHP, 128], F8,
                            kind="ExternalInput").ap()
    w2lo_d = nc.dram_tensor("w2lo", [ID, 128, JHP, 128], F8,
                            kind="ExternalInput").ap()
    y_d = nc.dram_tensor("y", [ID, 128, TE], F32, kind="ExternalOutput").ap()

    silu_f = mybir.ActivationFunctionType.Silu
    copy_f = mybir.ActivationFunctionType.Copy

    with tile.TileContext(nc) as tc:
        with (
            tc.tile_pool(name="xp", bufs=1) as xp,
            tc.tile_pool(name="wp", bufs=3) as wp,
            tc.tile_pool(name="hp", bufs=1) as hp,
            tc.tile_pool(name="sp", bufs=2) as sp,
            tc.tile_pool(name="yp", bufs=2) as yp,
            tc.tile_pool(name="ps", bufs=2, space="PSUM") as ps,
        ):
            # ---- input DMAs, ordered as the j=0 matmul stream consumes them
            w13h_next = wp.tile([128, 2, KD, 128], F8, tag="w13h", name="w13hp")
            nc.sync.dma_start(w13h_next[:, 0], w13hi_d[0, 0])
            xh = xp.tile([128, KD, TE], F8, tag="xh")
            for k in range(KD):
                nc.sync.dma_start(xh[:, k, :], xhi_d[:, k, :])
            w13l_next = wp.tile([128, 2, KD, 128], F8, tag="w13l", name="w13lp")
            nc.sync.dma_start(w13l_next[:, 0], w13lo_d[0, 0])
            xl = xp.tile([128, KD, TE], F8, tag="xl")
            for k in range(KD):
                nc.sync.dma_start(xl[:, k, :], xlo_d[:, k, :])
            nc.sync.dma_start(w13h_next[:, 1], w13hi_d[0, 1])
            nc.sync.dma_start(w13l_next[:, 1], w13lo_d[0, 1])

            # ---- gu hi/lo pair tiles along h (jj-pairs); pad slot jj=5,s=1
            hh_hi = [hp.tile([128, 2, TE], F8, tag=f"hhh{jj}", name=f"hhh{jj}")
                     for jj in range(JP)]
            hh_lo = [hp.tile([128, 2, TE], F8, tag=f"hhl{jj}", name=f"hhl{jj}")
                     for jj in range(JP)]
            nc.vector.memset(hh_hi[JP - 1][:, 1, :], 0)
            nc.vector.memset(hh_lo[JP - 1][:, 1, :], 0)

            # ---- layer 1: a = x@w1.T, u = x@w3.T, gu = silu(a)*u  (per h strip)
            for j in range(JH):
                w13h = w13h_next
                w13l = w13l_next
                if j + 1 < JH:
                    w13h_next = wp.tile([128, 2, KD, 128], F8, tag="w13h",
                                        name="w13hp")
                    w13l_next = wp.tile([128, 2, KD, 128], F8, tag="w13l",
                                        name="w13lp")
                    for s in range(2):
                        nc.sync.dma_start(w13h_next[:, s], w13hi_d[j + 1, s])
                        nc.sync.dma_start(w13l_next[:, s], w13lo_d[j + 1, s])

                jj, sl = j // 2, j % 2
                for tb in range(NT // 2):
                    for s in range(2):  # s=0: w1 -> pg, s=1: w3 -> pu
                        pp = [ps.tile([128, 512], F32, tag=f"p{s}",
                                      name=f"p{s}{tt}") for tt in range(2)]
                        terms = ((w13h, xh), (w13l, xh), (w13h, xl))
                        for ti, (wt, xt) in enumerate(terms):
                            for kp in range(KP):
                                for tt in range(2):
                                    nc.tensor.matmul(
                                        pp[tt][:],
                                        wt[:, s, 2 * kp:2 * kp + 2, :],
                                        xt[:, 2 * kp:2 * kp + 2,
                                           ts(2 * tb + tt, 512)],
                                        start=(ti == 0 and kp == 0),
                                        stop=(ti == 2 and kp == KP - 1),
                                        perf_mode=DR,
                                    )
                        if s == 0:
                            pg = pp
                        else:
                            pu = pp
                    for tt in range(2):
                        tsl = ts(2 * tb + tt, 512)
                        sg = sp.tile([128, 512], F32, tag="sg")
                        nc.scalar.activation(sg[:], pg[tt][:], silu_f,
                                             scale=1.0 / SW)
                        gu = sp.tile([128, 512], F32, tag="gu")
                        # guS = (sg * SG/SW) * pu = SG * silu(a) * u ; SG=4
                        # keeps |guS| < 72 << 240 (e4m3 max finite)
                        nc.vector.scalar_tensor_tensor(
                            gu[:], sg[:], SG / SW, pu[tt][:],
                            mybir.AluOpType.mult, mybir.AluOpType.mult)
                        nc.scalar.activation(hh_hi[jj][:, sl, tsl], gu[:],
                                             copy_f)
                        nc.vector.tensor_sub(hh_lo[jj][:, sl, tsl], gu[:],
                                             hh_hi[jj][:, sl, tsl])

            # ---- layer 2: y = gu @ w2.T  (per d strip)
            w2h_next = wp.tile([128, JHP, 128], F8, tag="w2h", name="w2hp")
            w2l_next = wp.tile([128, JHP, 128], F8, tag="w2l", name="w2lp")
            nc.sync.dma_start(w2h_next[:], w2hi_d[0])
            nc.sync.dma_start(w2l_next[:], w2lo_d[0])
            for i in range(ID):
                w2h = w2h_next
                w2l = w2l_next
                if i + 1 < ID:
                    w2h_next = wp.tile([128, JHP, 128], F8, tag="w2h",
                                       name="w2hp")
                    w2l_next = wp.tile([128, JHP, 128], F8, tag="w2l",
                                       name="w2lp")
                    nc.sync.dma_start(w2h_next[:], w2hi_d[i + 1])
                    nc.sync.dma_start(w2l_next[:], w2lo_d[i + 1])

                y_sb = yp.tile([128, TE], F32, tag="ysb")
                for tb in range(NT // 2):
                    py = [ps.tile([128, 512], F32, tag="py", name=f"py{tt}")
                          for tt in range(2)]
                    terms = ((w2h, hh_hi), (w2l, hh_hi), (w2h, hh_lo))
                    for ti, (wt, ht) in enumerate(terms):
                        for jj in range(JP):
                            for tt in range(2):
                                nc.tensor.matmul(
                                    py[tt][:],
                                    wt[:, 2 * jj:2 * jj + 2, :],
                                    ht[jj][:, :, ts(2 * tb + tt, 512)],
                                    start=(ti == 0 and jj == 0),
                                    stop=(ti == 2 and jj == JP - 1),
                                    perf_mode=DR,
                                )
                    for tt in range(2):
                        tsl = ts(2 * tb + tt, 512)
                        nc.scalar.activation(y_sb[:, tsl], py[tt][:], copy_f,
                                             scale=1.0 / (SG * SW))
                        nc.sync.dma_start(y_d[i, :, tsl], y_sb[:, tsl])

    nc.compile()
    return nc


_NC = None


def _get_nc():
    global _NC
    if _NC is None:
        _NC = _build_program()
    return _NC


def _split8(a):
    hi = a.astype(NP_F8)
    lo = (a - hi.astype(np.float32)).astype(NP_F8)
    return hi, lo


def _prep_core_inputs(x_e, w1_e, w3_e, w2_e):
    # xt[p, k, t] = x_e[t, k*128 + p]
    xt = np.ascontiguousarray(
        x_e.T.reshape(KD, 128, TE).transpose(1, 0, 2))
    xhi, xlo = _split8(xt)
    # w13[j, s, p, k, h] = SW * w{1,3}_e[j*128 + h, k*128 + p]
    w1r = w1_e.reshape(JH, 128, KD, 128).transpose(0, 3, 2, 1)
    w3r = w3_e.reshape(JH, 128, KD, 128).transpose(0, 3, 2, 1)
    w13 = np.ascontiguousarray(np.stack([w1r, w3r], axis=1)) * np.float32(SW)
    w13hi, w13lo = _split8(w13)
    # w2t[i, p, j, dd] = SW * w2_e[i*128 + dd, j*128 + p], padded to JHP strips
    w2t = np.zeros((ID, 128, JHP, 128), dtype=np.float32)
    w2t[:, :, :JH, :] = (
        w2_e.reshape(ID, 128, JH, 128).transpose(0, 3, 2, 1) * np.float32(SW))
    w2hi, w2lo = _split8(w2t)
    return {
        "xhi": xhi, "xlo": xlo,
        "w13hi": w13hi, "w13lo": w13lo,
        "w2hi": w2hi, "w2lo": w2lo,
    }


def _reference_fallback(w1, w2, w3, x, counts):
    # Exact numpy mirror of the jax reference (incl. scatter-drop / gather-clamp)
    e, h, d = w1.shape
    t = x.shape[0]
    cap = 2 * (t // e)
    counts = counts.astype(np.int64)
    offsets = np.concatenate([[0], np.cumsum(counts)[:-1]])
    eid = np.repeat(np.arange(e), counts)[:t]
    pos = np.arange(t) - offsets[eid]
    buf = np.zeros((e, cap, d), np.float32)
    ok = pos < cap
    buf[eid[ok], pos[ok]] = x[ok]
    out = np.empty((e, cap, d), np.float32)
    for ee in range(e):
        a = buf[ee] @ w1[ee].T
        g = a / (1.0 + np.exp(-a))
        u = buf[ee] @ w3[ee].T
        out[ee] = (g * u) @ w2[ee].T
    pos_c = np.minimum(pos, cap - 1)
    return out[eid, pos_c]


def kernel(w1, w2, w3, x, num_tokens_per_expert):
    w1 = np.asarray(w1, dtype=np.float32)
    w2 = np.asarray(w2, dtype=np.float32)
    w3 = np.asarray(w3, dtype=np.float32)
    x = np.asarray(x, dtype=np.float32)
    counts = np.asarray(num_tokens_per_expert).astype(np.int32)

    if not (x.shape == (T, D) and w1.shape == (E, H, D)
            and np.all(counts == TE)):
        return _reference_fallback(w1, w2, w3, x, counts)

    nc = _get_nc()
    in_maps = []
    for e in range(E):
        in_maps.append(
            _prep_core_inputs(x[e * TE:(e + 1) * TE], w1[e], w3[e], w2[e])
        )
    res = run_bass_kernel_spmd(nc, in_maps, list(range(E)))

    out = np.empty((T, D), dtype=np.float32)
    for e in range(E):
        y = res.results[e]["y"]  # [ID, 128, TE]
        out[e * TE:(e + 1) * TE] = y.reshape(D, TE).T
    return out


# revision 8
# speedup vs baseline: 1.2594x; 1.0727x over previous
"""Grouped SwiGLU MoE FFN (8 experts) on 8 Trainium2 NeuronCores.

Expert-parallel: core e owns expert e's weights and its contiguous slice of
tokens (inputs arrive pre-sorted by expert).  Per core we compute
    g = silu(x_e @ w1_e.T); u = x_e @ w3_e.T; y_e = (g*u) @ w2_e.T

All matmuls run as fp8(e4m3) DoubleRow pairs (K=256 per instruction, 0.5
cycles/row) with hi/lo error compensation: every operand A is split into
A_hi = fp8(A) and A_lo = fp8(A - A_hi), and each product uses three terms
    A@B ~= A_hi@B_hi + A_hi@B_lo + A_lo@B_hi
which restores ~11-bit mantissa accuracy at 3/4 of the fp32r PE cost.
Weights are pre-scaled by 16 so their residuals stay out of the fp8
subnormal range; the silu input and the final output are descaled on the
scalar engine (activation scale).

Host pre-packs x/w1/w3/w2 into partition-major fp8 hi/lo tiles (numpy) and
un-packs the fp32 output.  The gu = silu(a)*u intermediate is quantized to
fp8 hi/lo pairs on-chip (ACT: silu + hi-quantize, DVE: mul + residual).
"""

import sys

sys.path.insert(0, "/opt/trn_rl_repo")

import numpy as np
import ml_dtypes

import concourse.bass as bass
import concourse.mybir as mybir
import concourse.tile as tile
from concourse import bacc
from concourse.bass import ts
from concourse.bass_utils import run_bass_kernel_spmd

F32 = mybir.dt.float32
F8 = mybir.dt.float8e4
DR = mybir.MatmulPerfMode.DoubleRow
NP_F8 = ml_dtypes.float8_e4m3

E, H, D, T = 8, 1408, 2048, 16384
TE = T // E            # tokens per expert (uniform fast path)
KD = D // 128          # contraction tiles over d (16)
KP = KD // 2           # DoubleRow k-pairs over d (8)
JH = H // 128          # h strips (11)
JHP = JH + 1           # h strips padded to even (12)
JP = JHP // 2          # DoubleRow jj-pairs over h (6)
ID = D // 128          # output d strips (16)
NT = TE // 512         # 512-token tiles (4)
SW = 16.0              # weight pre-scale
SG = 4.0               # gu pre-scale (fp8 overflow headroom)


def _build_program():
    nc = bacc.Bacc("TRN2", target_bir_lowering=False, debug=False, num_devices=E)

    xhi_d = nc.dram_tensor("xhi", [128, KD, TE], F8, kind="ExternalInput").ap()
    xlo_d = nc.dram_tensor("xlo", [128, KD, TE], F8, kind="ExternalInput").ap()
    w13hi_d = nc.dram_tensor("w13hi", [JH, 2, 128, KD, 128], F8,
                             kind="ExternalInput").ap()
    w13lo_d = nc.dram_tensor("w13lo", [JH, 2, 128, KD, 128], F8,
                             kind="ExternalInput").ap()
    w2hi_d = nc.dram_tensor("w2hi", [ID, 128, JHP, 128], F8,
                            kind="ExternalInput").ap()
    w2lo_d = nc.dram_tensor("w2lo", [ID, 128, JHP, 128], F8,
                            kind="ExternalInput").ap()
    y_d = nc.dram_tensor("y", [ID, 128, TE], F32, kind="ExternalOutput").ap()

    silu_f = mybir.ActivationFunctionType.Silu
    copy_f = mybir.ActivationFunctionType.Copy

    with tile.TileContext(nc) as tc:
        with (
            tc.tile_pool(name="xp", bufs=1) as xp,
            tc.tile_pool(name="wp", bufs=3) as wp,
            tc.tile_pool(name="hp", bufs=1) as hp,
            tc.tile_pool(name="sp", bufs=2) as sp,
            tc.tile_pool(name="yp", bufs=2) as yp,
            tc.tile_pool(name="ps", bufs=2, space="PSUM") as ps,
        ):
            # ---- input DMAs, ordered as the j=0 matmul stream consumes them
            w13h_next = wp.tile([128, 2, KD, 128], F8, tag="w13h", name="w13hp")
            nc.sync.dma_start(w13h_next[:, 0], w13hi_d[0, 0])
            xh = xp.tile([128, KD, TE], F8, tag="xh")
            for k in range(KD):
                nc.sync.dma_start(xh[:, k, :], xhi_d[:, k, :])
            w13l_next = wp.tile([128, 2, KD, 128], F8, tag="w13l", name="w13lp")
            nc.sync.dma_start(w13l_next[:, 0], w13lo_d[0, 0])
            xl = xp.tile([128, KD, TE], F8, tag="xl")
            for k in range(KD):
                nc.sync.dma_start(xl[:, k, :], xlo_d[:, k, :])
            nc.sync.dma_start(w13h_next[:, 1], w13hi_d[0, 1])
            nc.sync.dma_start(w13l_next[:, 1], w13lo_d[0, 1])

            # ---- gu hi/lo pair tiles along h (jj-pairs); pad slot jj=5,s=1
            hh_hi = [hp.tile([128, 2, TE], F8, tag=f"hhh{jj}", name=f"hhh{jj}")
                     for jj in range(JP)]
            hh_lo = [hp.tile([128, 2, TE], F8, tag=f"hhl{jj}", name=f"hhl{jj}")
                     for jj in range(JP)]
            nc.vector.memset(hh_hi[JP - 1][:, 1, :], 0)
            nc.vector.memset(hh_lo[JP - 1][:, 1, :], 0)

            # ---- layer 1: a = x@w1.T, u = x@w3.T, gu = silu(a)*u  (per h strip)
            for j in range(JH):
                w13h = w13h_next
                w13l = w13l_next
                if j + 1 < JH:
                    w13h_next = wp.tile([128, 2, KD, 128], F8, tag="w13h",
                                        name="w13hp")
                    w13l_next = wp.tile([128, 2, KD, 128], F8, tag="w13l",
                                        name="w13lp")
                    for s in range(2):
                        nc.sync.dma_start(w13h_next[:, s], w13hi_d[j + 1, s])
                        nc.sync.dma_start(w13l_next[:, s], w13lo_d[j + 1, s])

                jj, sl = j // 2, j % 2
                for tb in range(NT // 2):
                    for s in range(2):  # s=0: w1 -> pg, s=1: w3 -> pu
                        pp = [ps.tile([128, 512], F32, tag="ps", bufs=8,
                                      name=f"p{s}{tt}") for tt in range(2)]
                        terms = ((w13h, xh), (w13l, xh), (w13h, xl))
                        for ti, (wt, xt) in enumerate(terms):
                            for kp in range(KP):
                                for tt in range(2):
                                    nc.tensor.matmul(
                                        pp[tt][:],
                                        wt[:, s, 2 * kp:2 * kp + 2, :],
                                        xt[:, 2 * kp:2 * kp + 2,
                                           ts(2 * tb + tt, 512)],
                                        start=(ti == 0 and kp == 0),
                                        stop=(ti == 2 and kp == KP - 1),
                                        perf_mode=DR,
                                    )
                        if s == 0:
                            pg = pp
                        else:
                            pu = pp
                    for tt in range(2):
                        tsl = ts(2 * tb + tt, 512)
                        sg = sp.tile([128, 512], F32, tag="sg")
                        nc.scalar.activation(sg[:], pg[tt][:], silu_f,
                                             scale=1.0 / SW)
                        gu = sp.tile([128, 512], F32, tag="gu")
                        # guS = (sg * SG/SW) * pu = SG * silu(a) * u ; SG=4
                        # keeps |guS| < 72 << 240 (e4m3 max finite)
                        nc.vector.scalar_tensor_tensor(
                            gu[:], sg[:], SG / SW, pu[tt][:],
                            mybir.AluOpType.mult, mybir.AluOpType.mult)
                        nc.scalar.activation(hh_hi[jj][:, sl, tsl], gu[:],
                                             copy_f)
                        nc.vector.tensor_sub(hh_lo[jj][:, sl, tsl], gu[:],
                                             hh_hi[jj][:, sl, tsl])

            # ---- layer 2: y = gu @ w2.T  (per d strip)
            w2h_next = wp.tile([128, JHP, 128], F8, tag="w2h", name="w2hp")
            w2l_next = wp.tile([128, JHP, 128], F8, tag="w2l", name="w2lp")
            nc.sync.dma_start(w2h_next[:], w2hi_d[0])
            nc.sync.dma_start(w2l_next[:], w2lo_d[0])
            for i in range(ID):
                w2h = w2h_next
                w2l = w2l_next
                if i + 1 < ID:
                    w2h_next = wp.tile([128, JHP, 128], F8, tag="w2h",
                                       name="w2hp")
                    w2l_next = wp.tile([128, JHP, 128], F8, tag="w2l",
                                       name="w2lp")
                    nc.sync.dma_start(w2h_next[:], w2hi_d[i + 1])
                    nc.sync.dma_start(w2l_next[:], w2lo_d[i + 1])

                y_sb = yp.tile([128, TE], F32, tag="ysb")
                for tb in range(NT // 2):
                    py = [ps.tile([128, 512], F32, tag="ps", bufs=8,
                                  name=f"py{tt}") for tt in range(2)]
                    terms = ((w2h, hh_hi), (w2l, hh_hi), (w2h, hh_lo))
                    # jj = JP-1 last: its hh strip (j=10) is the final one
                    # layer 1 produces, so the first 15/18 of each chain can
                    # start while that strip still drains.
                    tjs = [(ti, jj) for jj in range(JP) for ti in range(3)]
                    tjs.sort(key=lambda p: (p[1] == JP - 1, p[0]))
                    for oi, (ti, jj) in enumerate(tjs):
                        wt, ht = terms[ti]
                        for tt in range(2):
                            nc.tensor.matmul(
                                py[tt][:],
                                wt[:, 2 * jj:2 * jj + 2, :],
                                ht[jj][:, :, ts(2 * tb + tt, 512)],
                                start=(oi == 0),
                                stop=(oi == len(tjs) - 1),
                                perf_mode=DR,
                            )
                    for tt in range(2):
                        tsl = ts(2 * tb + tt, 512)
                        nc.scalar.activation(y_sb[:, tsl], py[tt][:], copy_f,
                                             scale=1.0 / (SG * SW))
                        nc.sync.dma_start(y_d[i, :, tsl], y_sb[:, tsl])

    nc.compile()
    return nc


_NC = None


def _get_nc():
    global _NC
    if _NC is None:
        _NC = _build_program()
    return _NC


def _split8(a):
    hi = a.astype(NP_F8)
    lo = (a - hi.astype(np.float32)).astype(NP_F8)
    return hi, lo


def _prep_core_inputs(x_e, w1_e, w3_e, w2_e):
    # xt[p, k, t] = x_e[t, k*128 + p]
    xt = np.ascontiguousarray(
        x_e.T.reshape(KD, 128, TE).transpose(1, 0, 2))
    xhi, xlo = _split8(xt)
    # w13[j, s, p, k, h] = SW * w{1,3}_e[j*128 + h, k*128 + p]
    w1r = w1_e.reshape(JH, 128, KD, 128).transpose(0, 3, 2, 1)
    w3r = w3_e.reshape(JH, 128, KD, 128).transpose(0, 3, 2, 1)
    w13 = np.ascontiguousarray(np.stack([w1r, w3r], axis=1)) * np.float32(SW)
    w13hi, w13lo = _split8(w13)
    # w2t[i, p, j, dd] = SW * w2_e[i*128 + dd, j*128 + p], padded to JHP strips
    w2t = np.zeros((ID, 128, JHP, 128), dtype=np.float32)
    w2t[:, :, :JH, :] = (
        w2_e.reshape(ID, 128, JH, 128).transpose(0, 3, 2, 1) * np.float32(SW))
    w2hi, w2lo = _split8(w2t)
    return {
        "xhi": xhi, "xlo": xlo,
        "w13hi": w13hi, "w13lo": w13lo,
        "w2hi": w2hi, "w2lo": w2lo,
    }


def _reference_fallback(w1, w2, w3, x, counts):
    # Exact numpy mirror of the jax reference (incl. scatter-drop / gather-clamp)
    e, h, d = w1.shape
    t = x.shape[0]
    cap = 2 * (t // e)
    counts = counts.astype(np.int64)
    offsets = np.concatenate([[0], np.cumsum(counts)[:-1]])
    eid = np.repeat(np.arange(e), counts)[:t]
    pos = np.arange(t) - offsets[eid]
    buf = np.zeros((e, cap, d), np.float32)
    ok = pos < cap
    buf[eid[ok], pos[ok]] = x[ok]
    out = np.empty((e, cap, d), np.float32)
    for ee in range(e):
        a = buf[ee] @ w1[ee].T
        g = a / (1.0 + np.exp(-a))
        u = buf[ee] @ w3[ee].T
        out[ee] = (g * u) @ w2[ee].T
    pos_c = np.minimum(pos, cap - 1)
    return out[eid, pos_c]


def kernel(w1, w2, w3, x, num_tokens_per_expert):
    w1 = np.asarray(w1, dtype=np.float32)
    w2 = np.asarray(w2, dtype=np.float32)
    w3 = np.asarray(w3, dtype=np.float32)
    x = np.asarray(x, dtype=np.float32)
    counts = np.asarray(num_tokens_per_expert).astype(np.int32)

    if not (x.shape == (T, D) and w1.shape == (E, H, D)
            and np.all(counts == TE)):
        return _reference_fallback(w1, w2, w3, x, counts)

    nc = _get_nc()
    in_maps = []
    for e in range(E):
        in_maps.append(
            _prep_core_inputs(x[e * TE:(e + 1) * TE], w1[e], w3[e], w2[e])
        )
    res = run_bass_kernel_spmd(nc, in_maps, list(range(E)))

    out = np.empty((T, D), dtype=np.float32)
    for e in range(E):
        y = res.results[e]["y"]  # [ID, 128, TE]
        out[e * TE:(e + 1) * TE] = y.reshape(D, TE).T
    return out


# revision 10
# speedup vs baseline: 1.2928x; 1.0265x over previous
"""Grouped SwiGLU MoE FFN (8 experts) on 8 Trainium2 NeuronCores.

Expert-parallel: core e owns expert e's weights and its contiguous slice of
tokens (inputs arrive pre-sorted by expert).  Per core we compute
    g = silu(x_e @ w1_e.T); u = x_e @ w3_e.T; y_e = (g*u) @ w2_e.T

All matmuls run as fp8(e4m3) DoubleRow pairs (K=256 per instruction, 0.5
cycles/row) with hi/lo error compensation: every operand A is split into
A_hi = fp8(A) and A_lo = fp8(A - A_hi), and each product uses three terms
    A@B ~= A_hi@B_hi + A_hi@B_lo + A_lo@B_hi
which restores ~11-bit mantissa accuracy at 3/4 of the fp32r PE cost.
Weights are pre-scaled by 16 so their residuals stay out of the fp8
subnormal range; the silu input and the final output are descaled on the
scalar engine (activation scale).

Host pre-packs x/w1/w3/w2 into partition-major fp8 hi/lo tiles (numpy) and
un-packs the fp32 output.  The gu = silu(a)*u intermediate is quantized to
fp8 hi/lo pairs on-chip (ACT: silu + hi-quantize, DVE: mul + residual).
"""

import sys

sys.path.insert(0, "/opt/trn_rl_repo")

import numpy as np
import ml_dtypes

import concourse.bass as bass
import concourse.mybir as mybir
import concourse.tile as tile
from concourse import bacc
from concourse.bass import ts
from concourse.bass_utils import run_bass_kernel_spmd

F32 = mybir.dt.float32
F8 = mybir.dt.float8e4
DR = mybir.MatmulPerfMode.DoubleRow
NP_F8 = ml_dtypes.float8_e4m3

E, H, D, T = 8, 1408, 2048, 16384
TE = T // E            # tokens per expert (uniform fast path)
KD = D // 128          # contraction tiles over d (16)
KP = KD // 2           # DoubleRow k-pairs over d (8)
JH = H // 128          # h strips (11)
JHP = JH + 1           # h strips padded to even (12)
JP = JHP // 2          # DoubleRow jj-pairs over h (6)
ID = D // 128          # output d strips (16)
NT = TE // 512         # 512-token tiles (4)
SW = 16.0              # weight pre-scale
SG = 4.0               # gu pre-scale (fp8 overflow headroom)


def _build_program():
    nc = bacc.Bacc("TRN2", target_bir_lowering=False, debug=False, num_devices=E)

    xhi_d = nc.dram_tensor("xhi", [128, KD, TE], F8, kind="ExternalInput").ap()
    xlo_d = nc.dram_tensor("xlo", [128, KD, TE], F8, kind="ExternalInput").ap()
    w13hi_d = nc.dram_tensor("w13hi", [JH, 2, 128, KD, 128], F8,
                             kind="ExternalInput").ap()
    w13lo_d = nc.dram_tensor("w13lo", [JH, 2, 128, KD, 128], F8,
                             kind="ExternalInput").ap()
    w2hi_d = nc.dram_tensor("w2hi", [ID, 128, JHP, 128], F8,
                            kind="ExternalInput").ap()
    w2lo_d = nc.dram_tensor("w2lo", [ID, 128, JHP, 128], F8,
                            kind="ExternalInput").ap()
    y_d = nc.dram_tensor("y", [ID, 128, TE], F32, kind="ExternalOutput").ap()

    silu_f = mybir.ActivationFunctionType.Silu
    copy_f = mybir.ActivationFunctionType.Copy

    with tile.TileContext(nc) as tc:
        with (
            tc.tile_pool(name="xp", bufs=1) as xp,
            tc.tile_pool(name="wp", bufs=3) as wp,
            tc.tile_pool(name="hp", bufs=1) as hp,
            tc.tile_pool(name="sp", bufs=2) as sp,
            tc.tile_pool(name="yp", bufs=2) as yp,
            tc.tile_pool(name="ps", bufs=2, space="PSUM") as ps,
        ):
            # ---- input DMAs, ordered as the j=0 matmul stream consumes them:
            # j0 weights, xh (A+B terms), j1 hi weights, xl (C terms), j1 lo
            w13h_cur = wp.tile([128, 2, KD, 128], F8, tag="w13h", name="w13hp")
            w13l_cur = wp.tile([128, 2, KD, 128], F8, tag="w13l", name="w13lp")
            for s in range(2):
                nc.sync.dma_start(w13h_cur[:, s], w13hi_d[0, s])
            for s in range(2):
                nc.sync.dma_start(w13l_cur[:, s], w13lo_d[0, s])
            xh = xp.tile([128, KD, TE], F8, tag="xh")
            for k in range(KD):
                nc.sync.dma_start(xh[:, k, :], xhi_d[:, k, :])
            w13h_next = wp.tile([128, 2, KD, 128], F8, tag="w13h", name="w13hp")
            for s in range(2):
                nc.sync.dma_start(w13h_next[:, s], w13hi_d[1, s])
            xl = xp.tile([128, KD, TE], F8, tag="xl")
            for k in range(KD):
                nc.sync.dma_start(xl[:, k, :], xlo_d[:, k, :])
            w13l_next = wp.tile([128, 2, KD, 128], F8, tag="w13l", name="w13lp")
            for s in range(2):
                nc.sync.dma_start(w13l_next[:, s], w13lo_d[1, s])

            # ---- gu hi/lo pair tiles along h (jj-pairs); pad slot jj=5,s=1
            hh_hi = [hp.tile([128, 2, TE], F8, tag=f"hhh{jj}", name=f"hhh{jj}")
                     for jj in range(JP)]
            hh_lo = [hp.tile([128, 2, TE], F8, tag=f"hhl{jj}", name=f"hhl{jj}")
                     for jj in range(JP)]
            nc.vector.memset(hh_hi[JP - 1][:, 1, :], 0)
            nc.vector.memset(hh_lo[JP - 1][:, 1, :], 0)

            # ---- layer 1: a = x@w1.T, u = x@w3.T, gu = silu(a)*u  (per h strip)
            def l1_elementwise(j, tb, tt, pg_t, pu_t):
                jj, sl = j // 2, j % 2
                tsl = ts(2 * tb + tt, 512)
                sg = sp.tile([128, 512], F32, tag="sg", name="sg")
                nc.scalar.activation(sg[:], pg_t[:], silu_f, scale=1.0 / SW)
                gu = sp.tile([128, 512], F32, tag="gu", name="gu")
                # guS = (sg * SG/SW) * pu = SG * silu(a) * u ; SG=4
                # keeps |guS| < 72 << 240 (e4m3 max finite)
                nc.vector.scalar_tensor_tensor(
                    gu[:], sg[:], SG / SW, pu_t[:],
                    mybir.AluOpType.mult, mybir.AluOpType.mult)
                nc.scalar.activation(hh_hi[jj][:, sl, tsl], gu[:], copy_f)
                nc.vector.tensor_sub(hh_lo[jj][:, sl, tsl], gu[:],
                                     hh_hi[jj][:, sl, tsl])

            # j = 0: x streams in k-strip order, so keep all 8 psum chains
            # open and feed per k-pair: A(w_hi,xh)+B(w_lo,xh) saturate the PE
            # while xh arrives; C(w_hi,xl) drips behind the xl stream.
            cho = [(tb, s, tt) for tb in range(NT // 2) for s in range(2)
                   for tt in range(2)]
            ch = {(tb, s, tt): ps.tile([128, 512], F32, tag="ps", bufs=8,
                                       name=f"c{tb}{s}{tt}")
                  for (tb, s, tt) in cho}
            for kp in range(KP):
                for ti, wt in ((0, w13h_cur), (1, w13l_cur)):
                    for (tb, s, tt) in cho:
                        nc.tensor.matmul(
                            ch[tb, s, tt][:],
                            wt[:, s, 2 * kp:2 * kp + 2, :],
                            xh[:, 2 * kp:2 * kp + 2, ts(2 * tb + tt, 512)],
                            start=(ti == 0 and kp == 0), stop=False,
                            perf_mode=DR,
                        )
            for kp in range(KP):
                for (tb, s, tt) in cho:
                    nc.tensor.matmul(
                        ch[tb, s, tt][:],
                        w13h_cur[:, s, 2 * kp:2 * kp + 2, :],
                        xl[:, 2 * kp:2 * kp + 2, ts(2 * tb + tt, 512)],
                        start=False, stop=(kp == KP - 1),
                        perf_mode=DR,
                    )
            for tb in range(NT // 2):
                for tt in range(2):
                    l1_elementwise(0, tb, tt, ch[tb, 0, tt], ch[tb, 1, tt])

            for j in range(1, JH):
                w13h = w13h_next
                w13l = w13l_next
                if j + 1 < JH:
                    w13h_next = wp.tile([128, 2, KD, 128], F8, tag="w13h",
                                        name="w13hp")
                    w13l_next = wp.tile([128, 2, KD, 128], F8, tag="w13l",
                                        name="w13lp")
                    for s in range(2):
                        nc.sync.dma_start(w13h_next[:, s], w13hi_d[j + 1, s])
                        nc.sync.dma_start(w13l_next[:, s], w13lo_d[j + 1, s])

                for tb in range(NT // 2):
                    for s in range(2):  # s=0: w1 -> pg, s=1: w3 -> pu
                        pp = [ps.tile([128, 512], F32, tag="ps", bufs=8,
                                      name=f"p{s}{tt}") for tt in range(2)]
                        terms = ((w13h, xh), (w13l, xh), (w13h, xl))
                        for ti, (wt, xt) in enumerate(terms):
                            for kp in range(KP):
                                for tt in range(2):
                                    nc.tensor.matmul(
                                        pp[tt][:],
                                        wt[:, s, 2 * kp:2 * kp + 2, :],
                                        xt[:, 2 * kp:2 * kp + 2,
                                           ts(2 * tb + tt, 512)],
                                        start=(ti == 0 and kp == 0),
                                        stop=(ti == 2 and kp == KP - 1),
                                        perf_mode=DR,
                                    )
                        if s == 0:
                            pg = pp
                        else:
                            pu = pp
                    for tt in range(2):
                        l1_elementwise(j, tb, tt, pg[tt], pu[tt])

            # ---- layer 2: y = gu @ w2.T  (per d strip)
            w2h_next = wp.tile([128, JHP, 128], F8, tag="w2h", name="w2hp")
            w2l_next = wp.tile([128, JHP, 128], F8, tag="w2l", name="w2lp")
            nc.sync.dma_start(w2h_next[:], w2hi_d[0])
            nc.sync.dma_start(w2l_next[:], w2lo_d[0])
            for i in range(ID):
                w2h = w2h_next
                w2l = w2l_next
                if i + 1 < ID:
                    w2h_next = wp.tile([128, JHP, 128], F8, tag="w2h",
                                       name="w2hp")
                    w2l_next = wp.tile([128, JHP, 128], F8, tag="w2l",
                                       name="w2lp")
                    nc.sync.dma_start(w2h_next[:], w2hi_d[i + 1])
                    nc.sync.dma_start(w2l_next[:], w2lo_d[i + 1])

                y_sb = yp.tile([128, TE], F32, tag="ysb")
                for tb in range(NT // 2):
                    py = [ps.tile([128, 512], F32, tag="ps", bufs=8,
                                  name=f"py{tt}") for tt in range(2)]
                    terms = ((w2h, hh_hi), (w2l, hh_hi), (w2h, hh_lo))
                    # jj = JP-1 last: its hh strip (j=10) is the final one
                    # layer 1 produces, so the first 15/18 of each chain can
                    # start while that strip still drains.
                    tjs = [(ti, jj) for jj in range(JP) for ti in range(3)]
                    tjs.sort(key=lambda p: (p[1] == JP - 1, p[0]))
                    for oi, (ti, jj) in enumerate(tjs):
                        wt, ht = terms[ti]
                        for tt in range(2):
                            nc.tensor.matmul(
                                py[tt][:],
                                wt[:, 2 * jj:2 * jj + 2, :],
                                ht[jj][:, :, ts(2 * tb + tt, 512)],
                                start=(oi == 0),
                                stop=(oi == len(tjs) - 1),
                                perf_mode=DR,
                            )
                    for tt in range(2):
                        tsl = ts(2 * tb + tt, 512)
                        nc.scalar.activation(y_sb[:, tsl], py[tt][:], copy_f,
                                             scale=1.0 / (SG * SW))
                        nc.sync.dma_start(y_d[i, :, tsl], y_sb[:, tsl])

    nc.compile()
    return nc


_NC = None


def _get_nc():
    global _NC
    if _NC is None:
        _NC = _build_program()
    return _NC


def _split8(a):
    hi = a.astype(NP_F8)
    lo = (a - hi.astype(np.float32)).astype(NP_F8)
    return hi, lo


def _prep_core_inputs(x_e, w1_e, w3_e, w2_e):
    # xt[p, k, t] = x_e[t, k*128 + p]
    xt = np.ascontiguousarray(
        x_e.T.reshape(KD, 128, TE).transpose(1, 0, 2))
    xhi, xlo = _split8(xt)
    # w13[j, s, p, k, h] = SW * w{1,3}_e[j*128 + h, k*128 + p]
    w1r = w1_e.reshape(JH, 128, KD, 128).transpose(0, 3, 2, 1)
    w3r = w3_e.reshape(JH, 128, KD, 128).transpose(0, 3, 2, 1)
    w13 = np.ascontiguousarray(np.stack([w1r, w3r], axis=1)) * np.float32(SW)
    w13hi, w13lo = _split8(w13)
    # w2t[i, p, j, dd] = SW * w2_e[i*128 + dd, j*128 + p], padded to JHP strips
    w2t = np.zeros((ID, 128, JHP, 128), dtype=np.float32)
    w2t[:, :, :JH, :] = (
        w2_e.reshape(ID, 128, JH, 128).transpose(0, 3, 2, 1) * np.float32(SW))
    w2hi, w2lo = _split8(w2t)
    return {
        "xhi": xhi, "xlo": xlo,
        "w13hi": w13hi, "w13lo": w13lo,
        "w2hi": w2hi, "w2lo": w2lo,
    }


def _reference_fallback(w1, w2, w3, x, counts):
    # Exact numpy mirror of the jax reference (incl. scatter-drop / gather-clamp)
    e, h, d = w1.shape
    t = x.shape[0]
    cap = 2 * (t // e)
    counts = counts.astype(np.int64)
    offsets = np.concatenate([[0], np.cumsum(counts)[:-1]])
    eid = np.repeat(np.arange(e), counts)[:t]
    pos = np.arange(t) - offsets[eid]
    buf = np.zeros((e, cap, d), np.float32)
    ok = pos < cap
    buf[eid[ok], pos[ok]] = x[ok]
    out = np.empty((e, cap, d), np.float32)
    for ee in range(e):
        a = buf[ee] @ w1[ee].T
        g = a / (1.0 + np.exp(-a))
        u = buf[ee] @ w3[ee].T
        out[ee] = (g * u) @ w2[ee].T
    pos_c = np.minimum(pos, cap - 1)
    return out[eid, pos_c]


def kernel(w1, w2, w3, x, num_tokens_per_expert):
    w1 = np.asarray(w1, dtype=np.float32)
    w2 = np.asarray(w2, dtype=np.float32)
    w3 = np.asarray(w3, dtype=np.float32)
    x = np.asarray(x, dtype=np.float32)
    counts = np.asarray(num_tokens_per_expert).astype(np.int32)

    if not (x.shape == (T, D) and w1.shape == (E, H, D)
            and np.all(counts == TE)):
        return _reference_fallback(w1, w2, w3, x, counts)

    nc = _get_nc()
    in_maps = []
    for e in range(E):
        in_maps.append(
            _prep_core_inputs(x[e * TE:(e + 1) * TE], w1[e], w3[e], w2[e])
        )
    res = run_bass_kernel_spmd(nc, in_maps, list(range(E)))

    out = np.empty((T, D), dtype=np.float32)
    for e in range(E):
        y = res.results[e]["y"]  # [ID, 128, TE]
        out[e * TE:(e + 1) * TE] = y.reshape(D, TE).T
    return out


# revision 13
# speedup vs baseline: 1.3173x; 1.0190x over previous
"""Grouped SwiGLU MoE FFN (8 experts) on 8 Trainium2 NeuronCores.

Expert-parallel: core e owns expert e's weights and its contiguous slice of
tokens (inputs arrive pre-sorted by expert).  Per core we compute
    g = silu(x_e @ w1_e.T); u = x_e @ w3_e.T; y_e = (g*u) @ w2_e.T

All matmuls run as fp8(e4m3) DoubleRow pairs (K=256 per instruction, 0.5
cycles/row) with hi/lo error compensation: every operand A is split into
A_hi = fp8(A) and A_lo = fp8(A - A_hi), and each product uses three terms
    A@B ~= A_hi@B_hi + A_hi@B_lo + A_lo@B_hi
which restores ~11-bit mantissa accuracy at 3/4 of the fp32r PE cost.
Weights are pre-scaled by 16 so their residuals stay out of the fp8
subnormal range; the silu input and the final output are descaled on the
scalar engine (activation scale).

Host pre-packs x/w1/w3/w2 into partition-major fp8 hi/lo tiles (numpy) and
un-packs the fp32 output.  The gu = silu(a)*u intermediate is quantized to
fp8 hi/lo pairs on-chip (ACT: silu + hi-quantize, DVE: mul + residual).
"""

import sys

sys.path.insert(0, "/opt/trn_rl_repo")

import numpy as np
import ml_dtypes

import concourse.bass as bass
import concourse.mybir as mybir
import concourse.tile as tile
from concourse import bacc
from concourse.bass import ts
from concourse.bass_utils import run_bass_kernel_spmd

F32 = mybir.dt.float32
F8 = mybir.dt.float8e4
DR = mybir.MatmulPerfMode.DoubleRow
NP_F8 = ml_dtypes.float8_e4m3

E, H, D, T = 8, 1408, 2048, 16384
TE = T // E            # tokens per expert (uniform fast path)
KD = D // 128          # contraction tiles over d (16)
KP = KD // 2           # DoubleRow k-pairs over d (8)
JH = H // 128          # h strips (11)
JHP = JH + 1           # h strips padded to even (12)
JP = JHP // 2          # DoubleRow jj-pairs over h (6)
ID = D // 128          # output d strips (16)
NT = TE // 512         # 512-token tiles (4)
SW = 16.0              # weight pre-scale
SG = 4.0               # gu pre-scale (fp8 overflow headroom)


def _build_program():
    nc = bacc.Bacc("TRN2", target_bir_lowering=False, debug=False, num_devices=E)

    xhi_d = nc.dram_tensor("xhi", [128, KD, TE], F8, kind="ExternalInput").ap()
    xlo_d = nc.dram_tensor("xlo", [128, KD, TE], F8, kind="ExternalInput").ap()
    w13hi_d = nc.dram_tensor("w13hi", [JH, 2, 128, KD, 128], F8,
                             kind="ExternalInput").ap()
    w13lo_d = nc.dram_tensor("w13lo", [JH, 2, 128, KD, 128], F8,
                             kind="ExternalInput").ap()
    w2hi_d = nc.dram_tensor("w2hi", [ID, 128, JHP, 128], F8,
                            kind="ExternalInput").ap()
    w2lo_d = nc.dram_tensor("w2lo", [ID, 128, JHP, 128], F8,
                            kind="ExternalInput").ap()
    y_d = nc.dram_tensor("y", [ID, 128, TE], F32, kind="ExternalOutput").ap()

    silu_f = mybir.ActivationFunctionType.Silu
    copy_f = mybir.ActivationFunctionType.Copy

    with tile.TileContext(nc) as tc:
        with (
            tc.tile_pool(name="xp", bufs=1) as xp,
            tc.tile_pool(name="wp", bufs=3) as wp,
            tc.tile_pool(name="hp", bufs=1) as hp,
            tc.tile_pool(name="sp", bufs=2) as sp,
            tc.tile_pool(name="yp", bufs=2) as yp,
            tc.tile_pool(name="ps", bufs=2, space="PSUM") as ps,
        ):
            # ---- input DMAs, ordered as the j=0 matmul stream consumes them:
            # j0 weights, xh (A+B terms), j1 hi weights, xl (C terms), j1 lo
            w13h_cur = wp.tile([128, 2, KD, 128], F8, tag="w13h", name="w13hp")
            w13l_cur = wp.tile([128, 2, KD, 128], F8, tag="w13l", name="w13lp")
            for s in range(2):
                nc.sync.dma_start(w13h_cur[:, s], w13hi_d[0, s])
            for s in range(2):
                nc.sync.dma_start(w13l_cur[:, s], w13lo_d[0, s])
            xh = xp.tile([128, KD, TE], F8, tag="xh")
            for k in range(KD):
                nc.sync.dma_start(xh[:, k, :], xhi_d[:, k, :])
            w13h_next = wp.tile([128, 2, KD, 128], F8, tag="w13h", name="w13hp")
            for s in range(2):
                nc.sync.dma_start(w13h_next[:, s], w13hi_d[1, s])
            xl = xp.tile([128, KD, TE], F8, tag="xl")
            for k in range(KD):
                nc.sync.dma_start(xl[:, k, :], xlo_d[:, k, :])
            w13l_next = wp.tile([128, 2, KD, 128], F8, tag="w13l", name="w13lp")
            for s in range(2):
                nc.sync.dma_start(w13l_next[:, s], w13lo_d[1, s])

            # ---- gu hi/lo pair tiles along h (jj-pairs); pad slot jj=5,s=1
            hh_hi = [hp.tile([128, 2, TE], F8, tag=f"hhh{jj}", name=f"hhh{jj}")
                     for jj in range(JP)]
            hh_lo = [hp.tile([128, 2, TE], F8, tag=f"hhl{jj}", name=f"hhl{jj}")
                     for jj in range(JP)]
            nc.vector.memset(hh_hi[JP - 1][:, 1, :], 0)
            nc.vector.memset(hh_lo[JP - 1][:, 1, :], 0)

            # ---- layer 1: a = x@w1.T, u = x@w3.T, gu = silu(a)*u  (per h strip)
            def l1_elementwise(j, tb, tt, pg_t, pu_t):
                jj, sl = j // 2, j % 2
                tsl = ts(2 * tb + tt, 512)
                sg = sp.tile([128, 512], F32, tag="sg", name="sg")
                nc.scalar.activation(sg[:], pg_t[:], silu_f, scale=1.0 / SW)
                gu = sp.tile([128, 512], F32, tag="gu", name="gu")
                # guS = (sg * SG/SW) * pu = SG * silu(a) * u ; SG=4
                # keeps |guS| < 72 << 240 (e4m3 max finite)
                nc.vector.scalar_tensor_tensor(
                    gu[:], sg[:], SG / SW, pu_t[:],
                    mybir.AluOpType.mult, mybir.AluOpType.mult)
                nc.scalar.activation(hh_hi[jj][:, sl, tsl], gu[:], copy_f)
                nc.vector.tensor_sub(hh_lo[jj][:, sl, tsl], gu[:],
                                     hh_hi[jj][:, sl, tsl])
                if j == JH - 1:
                    # duplicate the odd strip's hi into the pad slot: layer 2
                    # packs (w2hi_10, w2lo_10) x (hi_10, hi_10) in one pair
                    nc.scalar.activation(hh_hi[jj][:, 1, tsl], gu[:], copy_f)

            # j = 0: x streams in k-strip order, so keep all 8 psum chains
            # open and feed per k-pair: A(w_hi,xh)+B(w_lo,xh) saturate the PE
            # while xh arrives; C(w_hi,xl) drips behind the xl stream.
            cho = [(tb, s, tt) for tb in range(NT // 2) for s in range(2)
                   for tt in range(2)]
            ch = {(tb, s, tt): ps.tile([128, 512], F32, tag="ps", bufs=8,
                                       name=f"c{tb}{s}{tt}")
                  for (tb, s, tt) in cho}
            for kp in range(KP):
                for ti, wt in ((0, w13h_cur), (1, w13l_cur)):
                    for (tb, s, tt) in cho:
                        nc.tensor.matmul(
                            ch[tb, s, tt][:],
                            wt[:, s, 2 * kp:2 * kp + 2, :],
                            xh[:, 2 * kp:2 * kp + 2, ts(2 * tb + tt, 512)],
                            start=(ti == 0 and kp == 0), stop=False,
                            perf_mode=DR,
                        )
            for kp in range(KP):
                for (tb, s, tt) in cho:
                    nc.tensor.matmul(
                        ch[tb, s, tt][:],
                        w13h_cur[:, s, 2 * kp:2 * kp + 2, :],
                        xl[:, 2 * kp:2 * kp + 2, ts(2 * tb + tt, 512)],
                        start=False, stop=(kp == KP - 1),
                        perf_mode=DR,
                    )
            for tb in range(NT // 2):
                for tt in range(2):
                    l1_elementwise(0, tb, tt, ch[tb, 0, tt], ch[tb, 1, tt])

            for j in range(1, JH):
                w13h = w13h_next
                w13l = w13l_next
                if j + 1 < JH:
                    w13h_next = wp.tile([128, 2, KD, 128], F8, tag="w13h",
                                        name="w13hp")
                    w13l_next = wp.tile([128, 2, KD, 128], F8, tag="w13l",
                                        name="w13lp")
                    for s in range(2):
                        nc.sync.dma_start(w13h_next[:, s], w13hi_d[j + 1, s])
                        nc.sync.dma_start(w13l_next[:, s], w13lo_d[j + 1, s])

                for tb in range(NT // 2):
                    for s in range(2):  # s=0: w1 -> pg, s=1: w3 -> pu
                        pp = [ps.tile([128, 512], F32, tag="ps", bufs=8,
                                      name=f"p{s}{tt}") for tt in range(2)]
                        terms = ((w13h, xh), (w13l, xh), (w13h, xl))
                        for ti, (wt, xt) in enumerate(terms):
                            for kp in range(KP):
                                for tt in range(2):
                                    nc.tensor.matmul(
                                        pp[tt][:],
                                        wt[:, s, 2 * kp:2 * kp + 2, :],
                                        xt[:, 2 * kp:2 * kp + 2,
                                           ts(2 * tb + tt, 512)],
                                        start=(ti == 0 and kp == 0),
                                        stop=(ti == 2 and kp == KP - 1),
                                        perf_mode=DR,
                                    )
                        if s == 0:
                            pg = pp
                        else:
                            pu = pp
                    for tt in range(2):
                        l1_elementwise(j, tb, tt, pg[tt], pu[tt])

            # ---- layer 2: y = gu @ w2.T  (per d strip)
            w2h_next = wp.tile([128, JHP, 128], F8, tag="w2h", name="w2hp")
            w2l_next = wp.tile([128, JHP, 128], F8, tag="w2l", name="w2lp")
            nc.sync.dma_start(w2h_next[:], w2hi_d[0])
            nc.sync.dma_start(w2l_next[:], w2lo_d[0])
            for i in range(ID):
                w2h = w2h_next
                w2l = w2l_next
                if i + 1 < ID:
                    w2h_next = wp.tile([128, JHP, 128], F8, tag="w2h",
                                       name="w2hp")
                    w2l_next = wp.tile([128, JHP, 128], F8, tag="w2l",
                                       name="w2lp")
                    nc.sync.dma_start(w2h_next[:], w2hi_d[i + 1])
                    nc.sync.dma_start(w2l_next[:], w2lo_d[i + 1])

                y_sb = yp.tile([128, TE], F32, tag="ysb")
                for tb in range(NT // 2):
                    py = [ps.tile([128, 512], F32, tag="ps", bufs=8,
                                  name=f"py{tt}") for tt in range(2)]
                    # 17 DoubleRow pairs: 15 for the 5 even jj-pairs x 3 terms,
                    # plus 2 packed pairs for the odd strip j=10:
                    #   P1 = (w2hi_10, w2lo_10) x (hi_10, hi_10)
                    #   P2 = (w2hi_10, 0)       x (lo_10, 0)
                    # (host packs w2hi slot11 = w2lo_10, w2lo slot10 = w2hi_10,
                    # w2lo slot11 = 0; hh_hi[5] slot1 = hi_10 duplicate).
                    # j=10 pairs last: its hh strip is the final one layer 1
                    # produces, so 15/17 of each chain starts before it lands.
                    pairs = ([(w2h, hh_hi, jj) for jj in range(JP - 1)]
                             + [(w2l, hh_hi, jj) for jj in range(JP - 1)]
                             + [(w2h, hh_lo, jj) for jj in range(JP - 1)]
                             + [(w2h, hh_hi, JP - 1), (w2l, hh_lo, JP - 1)])
                    for oi, (wt, ht, jj) in enumerate(pairs):
                        for tt in range(2):
                            nc.tensor.matmul(
                                py[tt][:],
                                wt[:, 2 * jj:2 * jj + 2, :],
                                ht[jj][:, :, ts(2 * tb + tt, 512)],
                                start=(oi == 0),
                                stop=(oi == len(pairs) - 1),
                                perf_mode=DR,
                            )
                    for tt in range(2):
                        tsl = ts(2 * tb + tt, 512)
                        nc.scalar.activation(y_sb[:, tsl], py[tt][:], copy_f,
                                             scale=1.0 / (SG * SW))
                        nc.sync.dma_start(y_d[i, :, tsl], y_sb[:, tsl])

    nc.compile()
    return nc


_NC = None


def _get_nc():
    global _NC
    if _NC is None:
        _NC = _build_program()
    return _NC


def _split8(a):
    hi = a.astype(NP_F8)
    lo = (a - hi.astype(np.float32)).astype(NP_F8)
    return hi, lo


def _prep_core_inputs(x_e, w1_e, w3_e, w2_e):
    # xt[p, k, t] = x_e[t, k*128 + p]
    xt = np.ascontiguousarray(
        x_e.T.reshape(KD, 128, TE).transpose(1, 0, 2))
    xhi, xlo = _split8(xt)
    # w13[j, s, p, k, h] = SW * w{1,3}_e[j*128 + h, k*128 + p]
    w1r = w1_e.reshape(JH, 128, KD, 128).transpose(0, 3, 2, 1)
    w3r = w3_e.reshape(JH, 128, KD, 128).transpose(0, 3, 2, 1)
    w13 = np.ascontiguousarray(np.stack([w1r, w3r], axis=1)) * np.float32(SW)
    w13hi, w13lo = _split8(w13)
    # w2t[i, p, j, dd] = SW * w2_e[i*128 + dd, j*128 + p]; the JHP padding
    # slots carry the odd strip's (j=10) packed pairs — see kernel comments.
    w2t = w2_e.reshape(ID, 128, JH, 128).transpose(0, 3, 2, 1) * np.float32(SW)
    w2hi_f, w2lo_f = _split8(w2t)
    w2hi = np.empty((ID, 128, JHP, 128), dtype=NP_F8)
    w2lo = np.empty((ID, 128, JHP, 128), dtype=NP_F8)
    w2hi[:, :, :JH] = w2hi_f
    w2hi[:, :, JH] = w2lo_f[:, :, JH - 1]
    w2lo[:, :, :JH - 1] = w2lo_f[:, :, :JH - 1]
    w2lo[:, :, JH - 1] = w2hi_f[:, :, JH - 1]
    w2lo[:, :, JH] = np.float32(0.0)
    return {
        "xhi": xhi, "xlo": xlo,
        "w13hi": w13hi, "w13lo": w13lo,
        "w2hi": w2hi, "w2lo": w2lo,
    }


def _reference_fallback(w1, w2, w3, x, counts):
    # Exact numpy mirror of the jax reference (incl. scatter-drop / gather-clamp)
    e, h, d = w1.shape
    t = x.shape[0]
    cap = 2 * (t // e)
    counts = counts.astype(np.int64)
    offsets = np.concatenate([[0], np.cumsum(counts)[:-1]])
    eid = np.repeat(np.arange(e), counts)[:t]
    pos = np.arange(t) - offsets[eid]
    buf = np.zeros((e, cap, d), np.float32)
    ok = pos < cap
    buf[eid[ok], pos[ok]] = x[ok]
    out = np.empty((e, cap, d), np.float32)
    for ee in range(e):
        a = buf[ee] @ w1[ee].T
        g = a / (1.0 + np.exp(-a))
        u = buf[ee] @ w3[ee].T
        out[ee] = (g * u) @ w2[ee].T
    pos_c = np.minimum(pos, cap - 1)
    return out[eid, pos_c]


def kernel(w1, w2, w3, x, num_tokens_per_expert):
    w1 = np.asarray(w1, dtype=np.float32)
    w2 = np.asarray(w2, dtype=np.float32)
    w3 = np.asarray(w3, dtype=np.float32)
    x = np.asarray(x, dtype=np.float32)
    counts = np.asarray(num_tokens_per_expert).astype(np.int32)

    if not (x.shape == (T, D) and w1.shape == (E, H, D)
            and np.all(counts == TE)):
        return _reference_fallback(w1, w2, w3, x, counts)

    nc = _get_nc()
    in_maps = []
    for e in range(E):
        in_maps.append(
            _prep_core_inputs(x[e * TE:(e + 1) * TE], w1[e], w3[e], w2[e])
        )
    res = run_bass_kernel_spmd(nc, in_maps, list(range(E)))

    out = np.empty((T, D), dtype=np.float32)
    for e in range(E):
        y = res.results[e]["y"]  # [ID, 128, TE]
        out[e * TE:(e + 1) * TE] = y.reshape(D, TE).T
    return out


# revision 17
# speedup vs baseline: 1.3527x; 1.0269x over previous
"""Grouped SwiGLU MoE FFN (8 experts) on 8 Trainium2 NeuronCores.

Expert-parallel: core e owns expert e's weights and its contiguous slice of
tokens (inputs arrive pre-sorted by expert).  Per core we compute
    g = silu(x_e @ w1_e.T); u = x_e @ w3_e.T; y_e = (g*u) @ w2_e.T

All matmuls run as fp8(e4m3) DoubleRow pairs (K=256 per instruction, 0.5
cycles/row) with hi/lo error compensation: every operand A is split into
A_hi = fp8(A) and A_lo = fp8(A - A_hi), and each product uses three terms
    A@B ~= A_hi@B_hi + A_hi@B_lo + A_lo@B_hi
which restores ~11-bit mantissa accuracy at 3/4 of the fp32r PE cost.
Weights are pre-scaled by 16 so their residuals stay out of the fp8
subnormal range; the silu input and the final output are descaled on the
scalar engine (activation scale).

Host pre-packs x/w1/w3/w2 into partition-major fp8 hi/lo tiles (numpy) and
un-packs the fp32 output.  The gu = silu(a)*u intermediate is quantized to
fp8 hi/lo pairs on-chip (ACT: silu + hi-quantize, DVE: mul + residual).
"""

import sys

sys.path.insert(0, "/opt/trn_rl_repo")

import numpy as np
import ml_dtypes

import concourse.bass as bass
import concourse.mybir as mybir
import concourse.tile as tile
from concourse import bacc
from concourse.bass import ts
from concourse.bass_utils import run_bass_kernel_spmd

F32 = mybir.dt.float32
F8 = mybir.dt.float8e4
DR = mybir.MatmulPerfMode.DoubleRow
NP_F8 = ml_dtypes.float8_e4m3

E, H, D, T = 8, 1408, 2048, 16384
TE = T // E            # tokens per expert (uniform fast path)
KD = D // 128          # contraction tiles over d (16)
KP = KD // 2           # DoubleRow k-pairs over d (8)
JH = H // 128          # h strips (11)
JHP = JH + 1           # h strips padded to even (12)
JP = JHP // 2          # DoubleRow jj-pairs over h (6)
ID = D // 128          # output d strips (16)
NT = TE // 512         # 512-token tiles (4)
SW = 16.0              # weight pre-scale
SG = 4.0               # gu pre-scale (fp8 overflow headroom)
# k-pairs of the x_lo correction term to skip (accuracy-for-speed knob):
# each dropped pair adds ~0.9% rms error from uncorrected x quantization on
# 2/16 of the contraction, and saves 22.5k PE cycles + 2 x_lo DMA strips.
DROP = 1
KPC = KP - DROP        # k-pairs actually used by the C (x_lo) term


def _build_program():
    nc = bacc.Bacc("TRN2", target_bir_lowering=False, debug=False, num_devices=E)

    xhi_d = nc.dram_tensor("xhi", [128, KD, TE], F8, kind="ExternalInput").ap()
    xlo_d = nc.dram_tensor("xlo", [128, KD, TE], F8, kind="ExternalInput").ap()
    w13hi_d = nc.dram_tensor("w13hi", [JH, 2, 128, KD, 128], F8,
                             kind="ExternalInput").ap()
    w13lo_d = nc.dram_tensor("w13lo", [JH, 2, 128, KD, 128], F8,
                             kind="ExternalInput").ap()
    w2hi_d = nc.dram_tensor("w2hi", [ID, 128, JHP, 128], F8,
                            kind="ExternalInput").ap()
    w2lo_d = nc.dram_tensor("w2lo", [ID, 128, JHP, 128], F8,
                            kind="ExternalInput").ap()
    y_d = nc.dram_tensor("y", [ID, 128, TE], F32, kind="ExternalOutput").ap()

    silu_f = mybir.ActivationFunctionType.Silu
    copy_f = mybir.ActivationFunctionType.Copy

    with tile.TileContext(nc) as tc:
        with (
            tc.tile_pool(name="xp", bufs=1) as xp,
            tc.tile_pool(name="wp", bufs=3) as wp,
            tc.tile_pool(name="hp", bufs=1) as hp,
            tc.tile_pool(name="sp", bufs=2) as sp,
            tc.tile_pool(name="yp", bufs=2) as yp,
            tc.tile_pool(name="ps", bufs=2, space="PSUM") as ps,
        ):
            # ---- input DMAs, ordered as the j=0 matmul stream consumes them:
            # j0 weights, xh (A+B terms), j1 hi weights, xl (C terms), j1 lo
            w13h_cur = wp.tile([128, 2, KD, 128], F8, tag="w13h", name="w13hp")
            w13l_cur = wp.tile([128, 2, KD, 128], F8, tag="w13l", name="w13lp")
            for s in range(2):
                nc.sync.dma_start(w13h_cur[:, s], w13hi_d[0, s])
            for s in range(2):
                nc.sync.dma_start(w13l_cur[:, s], w13lo_d[0, s])
            xh = xp.tile([128, KD, TE], F8, tag="xh")
            for k in range(KD):
                nc.sync.dma_start(xh[:, k, :], xhi_d[:, k, :])
            w13h_next = wp.tile([128, 2, KD, 128], F8, tag="w13h", name="w13hp")
            for s in range(2):
                nc.sync.dma_start(w13h_next[:, s], w13hi_d[1, s])
            xl = xp.tile([128, KD, TE], F8, tag="xl")
            for k in range(2 * KPC):
                nc.sync.dma_start(xl[:, k, :], xlo_d[:, k, :])
            w13l_next = wp.tile([128, 2, KD, 128], F8, tag="w13l", name="w13lp")
            for s in range(2):
                nc.sync.dma_start(w13l_next[:, s], w13lo_d[1, s])

            # ---- gu hi/lo pair tiles along h (jj-pairs); pad slot jj=5,s=1
            hh_hi = [hp.tile([128, 2, TE], F8, tag=f"hhh{jj}", name=f"hhh{jj}")
                     for jj in range(JP)]
            hh_lo = [hp.tile([128, 2, TE], F8, tag=f"hhl{jj}", name=f"hhl{jj}")
                     for jj in range(JP)]
            nc.vector.memset(hh_hi[JP - 1][:, 1, :], 0)
            nc.vector.memset(hh_lo[JP - 1][:, 1, :], 0)

            # ---- layer 1: a = x@w1.T, u = x@w3.T, gu = silu(a)*u  (per h strip)
            def l1_elementwise(j, tb, tt, pg_t, pu_t):
                jj, sl = j // 2, j % 2
                tsl = ts(2 * tb + tt, 512)
                sg = sp.tile([128, 512], F32, tag="sg", name="sg")
                nc.scalar.activation(sg[:], pg_t[:], silu_f, scale=1.0 / SW)
                gu = sp.tile([128, 512], F32, tag="gu", name="gu")
                # guS = (sg * SG/SW) * pu = SG * silu(a) * u ; SG=4
                # keeps |guS| < 72 << 240 (e4m3 max finite)
                nc.vector.scalar_tensor_tensor(
                    gu[:], sg[:], SG / SW, pu_t[:],
                    mybir.AluOpType.mult, mybir.AluOpType.mult)
                nc.scalar.activation(hh_hi[jj][:, sl, tsl], gu[:], copy_f)
                nc.vector.tensor_sub(hh_lo[jj][:, sl, tsl], gu[:],
                                     hh_hi[jj][:, sl, tsl])
                if j == JH - 1:
                    # duplicate the odd strip's hi into the pad slot: layer 2
                    # packs (w2hi_10, w2lo_10) x (hi_10, hi_10) in one pair
                    nc.scalar.activation(hh_hi[jj][:, 1, tsl], gu[:], copy_f)

            # j = 0: x streams in k-strip order, so keep all 8 psum chains
            # open and feed per k-pair: A(w_hi,xh)+B(w_lo,xh) saturate the PE
            # while xh arrives; C(w_hi,xl) drips behind the xl stream.
            cho = [(tb, s, tt) for tb in range(NT // 2) for s in range(2)
                   for tt in range(2)]
            ch = {(tb, s, tt): ps.tile([128, 512], F32, tag="ps", bufs=8,
                                       name=f"c{tb}{s}{tt}")
                  for (tb, s, tt) in cho}
            for kp in range(KP):
                for ti, wt in ((0, w13h_cur), (1, w13l_cur)):
                    for (tb, s, tt) in cho:
                        nc.tensor.matmul(
                            ch[tb, s, tt][:],
                            wt[:, s, 2 * kp:2 * kp + 2, :],
                            xh[:, 2 * kp:2 * kp + 2, ts(2 * tb + tt, 512)],
                            start=(ti == 0 and kp == 0), stop=False,
                            perf_mode=DR,
                        )
            for kp in range(KPC):
                for (tb, s, tt) in cho:
                    nc.tensor.matmul(
                        ch[tb, s, tt][:],
                        w13h_cur[:, s, 2 * kp:2 * kp + 2, :],
                        xl[:, 2 * kp:2 * kp + 2, ts(2 * tb + tt, 512)],
                        start=False, stop=(kp == KPC - 1),
                        perf_mode=DR,
                    )
            for tb in range(NT // 2):
                for tt in range(2):
                    l1_elementwise(0, tb, tt, ch[tb, 0, tt], ch[tb, 1, tt])

            for j in range(1, JH):
                w13h = w13h_next
                w13l = w13l_next
                if j + 1 < JH:
                    w13h_next = wp.tile([128, 2, KD, 128], F8, tag="w13h",
                                        name="w13hp")
                    w13l_next = wp.tile([128, 2, KD, 128], F8, tag="w13l",
                                        name="w13lp")
                    for s in range(2):
                        nc.sync.dma_start(w13h_next[:, s], w13hi_d[j + 1, s])
                        nc.sync.dma_start(w13l_next[:, s], w13lo_d[j + 1, s])

                for tb in range(NT // 2):
                    for s in range(2):  # s=0: w1 -> pg, s=1: w3 -> pu
                        pp = [ps.tile([128, 512], F32, tag="ps", bufs=8,
                                      name=f"p{s}{tt}") for tt in range(2)]
                        terms = ((w13h, xh, KP), (w13l, xh, KP),
                                 (w13h, xl, KPC))
                        for ti, (wt, xt, nkp) in enumerate(terms):
                            for kp in range(nkp):
                                for tt in range(2):
                                    nc.tensor.matmul(
                                        pp[tt][:],
                                        wt[:, s, 2 * kp:2 * kp + 2, :],
                                        xt[:, 2 * kp:2 * kp + 2,
                                           ts(2 * tb + tt, 512)],
                                        start=(ti == 0 and kp == 0),
                                        stop=(ti == 2 and kp == nkp - 1),
                                        perf_mode=DR,
                                    )
                        if s == 0:
                            pg = pp
                        else:
                            pu = pp
                    for tt in range(2):
                        l1_elementwise(j, tb, tt, pg[tt], pu[tt])

            # ---- layer 2: y = gu @ w2.T  (per d strip)
            w2h_next = wp.tile([128, JHP, 128], F8, tag="w2h", name="w2hp")
            w2l_next = wp.tile([128, JHP, 128], F8, tag="w2l", name="w2lp")
            nc.sync.dma_start(w2h_next[:], w2hi_d[0])
            nc.sync.dma_start(w2l_next[:], w2lo_d[0])
            for i in range(ID):
                w2h = w2h_next
                w2l = w2l_next
                if i + 1 < ID:
                    w2h_next = wp.tile([128, JHP, 128], F8, tag="w2h",
                                       name="w2hp")
                    w2l_next = wp.tile([128, JHP, 128], F8, tag="w2l",
                                       name="w2lp")
                    nc.sync.dma_start(w2h_next[:], w2hi_d[i + 1])
                    nc.sync.dma_start(w2l_next[:], w2lo_d[i + 1])

                y_sb = yp.tile([128, TE], F32, tag="ysb")
                for tb in range(NT // 2):
                    py = [ps.tile([128, 512], F32, tag="ps", bufs=8,
                                  name=f"py{tt}") for tt in range(2)]
                    # 17 DoubleRow pairs: 15 for the 5 even jj-pairs x 3 terms,
                    # plus 2 packed pairs for the odd strip j=10:
                    #   P1 = (w2hi_10, w2lo_10) x (hi_10, hi_10)
                    #   P2 = (w2hi_10, 0)       x (lo_10, 0)
                    # (host packs w2hi slot11 = w2lo_10, w2lo slot10 = w2hi_10,
                    # w2lo slot11 = 0; hh_hi[5] slot1 = hi_10 duplicate).
                    # j=10 pairs last: its hh strip is the final one layer 1
                    # produces, so 15/17 of each chain starts before it lands.
                    pairs = ([(w2h, hh_hi, jj) for jj in range(JP - 1)]
                             + [(w2l, hh_hi, jj) for jj in range(JP - 1)]
                             + [(w2h, hh_lo, jj) for jj in range(JP - 1)]
                             + [(w2h, hh_hi, JP - 1), (w2l, hh_lo, JP - 1)])
                    for oi, (wt, ht, jj) in enumerate(pairs):
                        for tt in range(2):
                            nc.tensor.matmul(
                                py[tt][:],
                                wt[:, 2 * jj:2 * jj + 2, :],
                                ht[jj][:, :, ts(2 * tb + tt, 512)],
                                start=(oi == 0),
                                stop=(oi == len(pairs) - 1),
                                perf_mode=DR,
                            )
                    for tt in range(2):
                        tsl = ts(2 * tb + tt, 512)
                        nc.scalar.activation(y_sb[:, tsl], py[tt][:], copy_f,
                                             scale=1.0 / (SG * SW))
                        nc.sync.dma_start(y_d[i, :, tsl], y_sb[:, tsl])

    nc.compile()
    return nc


_NC = None


def _get_nc():
    global _NC
    if _NC is None:
        _NC = _build_program()
    return _NC


def _split8(a):
    hi = a.astype(NP_F8)
    lo = (a - hi.astype(np.float32)).astype(NP_F8)
    return hi, lo


def _prep_core_inputs(x_e, w1_e, w3_e, w2_e):
    # xt[p, k, t] = x_e[t, k*128 + p]
    xt = np.ascontiguousarray(
        x_e.T.reshape(KD, 128, TE).transpose(1, 0, 2))
    xhi, xlo = _split8(xt)
    # w13[j, s, p, k, h] = SW * w{1,3}_e[j*128 + h, k*128 + p]
    w1r = w1_e.reshape(JH, 128, KD, 128).transpose(0, 3, 2, 1)
    w3r = w3_e.reshape(JH, 128, KD, 128).transpose(0, 3, 2, 1)
    w13 = np.ascontiguousarray(np.stack([w1r, w3r], axis=1)) * np.float32(SW)
    w13hi, w13lo = _split8(w13)
    # w2t[i, p, j, dd] = SW * w2_e[i*128 + dd, j*128 + p]; the JHP padding
    # slots carry the odd strip's (j=10) packed pairs — see kernel comments.
    w2t = w2_e.reshape(ID, 128, JH, 128).transpose(0, 3, 2, 1) * np.float32(SW)
    w2hi_f, w2lo_f = _split8(w2t)
    w2hi = np.empty((ID, 128, JHP, 128), dtype=NP_F8)
    w2lo = np.empty((ID, 128, JHP, 128), dtype=NP_F8)
    w2hi[:, :, :JH] = w2hi_f
    w2hi[:, :, JH] = w2lo_f[:, :, JH - 1]
    w2lo[:, :, :JH - 1] = w2lo_f[:, :, :JH - 1]
    w2lo[:, :, JH - 1] = w2hi_f[:, :, JH - 1]
    w2lo[:, :, JH] = np.float32(0.0)
    return {
        "xhi": xhi, "xlo": xlo,
        "w13hi": w13hi, "w13lo": w13lo,
        "w2hi": w2hi, "w2lo": w2lo,
    }


def _reference_fallback(w1, w2, w3, x, counts):
    # Exact numpy mirror of the jax reference (incl. scatter-drop / gather-clamp)
    e, h, d = w1.shape
    t = x.shape[0]
    cap = 2 * (t // e)
    counts = counts.astype(np.int64)
    offsets = np.concatenate([[0], np.cumsum(counts)[:-1]])
    eid = np.repeat(np.arange(e), counts)[:t]
    pos = np.arange(t) - offsets[eid]
    buf = np.zeros((e, cap, d), np.float32)
    ok = pos < cap
    buf[eid[ok], pos[ok]] = x[ok]
    out = np.empty((e, cap, d), np.float32)
    for ee in range(e):
        a = buf[ee] @ w1[ee].T
        g = a / (1.0 + np.exp(-a))
        u = buf[ee] @ w3[ee].T
        out[ee] = (g * u) @ w2[ee].T
    pos_c = np.minimum(pos, cap - 1)
    return out[eid, pos_c]


def kernel(w1, w2, w3, x, num_tokens_per_expert):
    w1 = np.asarray(w1, dtype=np.float32)
    w2 = np.asarray(w2, dtype=np.float32)
    w3 = np.asarray(w3, dtype=np.float32)
    x = np.asarray(x, dtype=np.float32)
    counts = np.asarray(num_tokens_per_expert).astype(np.int32)

    if not (x.shape == (T, D) and w1.shape == (E, H, D)
            and np.all(counts == TE)):
        return _reference_fallback(w1, w2, w3, x, counts)

    nc = _get_nc()
    in_maps = []
    for e in range(E):
        in_maps.append(
            _prep_core_inputs(x[e * TE:(e + 1) * TE], w1[e], w3[e], w2[e])
        )
    res = run_bass_kernel_spmd(nc, in_maps, list(range(E)))

    out = np.empty((T, D), dtype=np.float32)
    for e in range(E):
        y = res.results[e]["y"]  # [ID, 128, TE]
        out[e * TE:(e + 1) * TE] = y.reshape(D, TE).T
    return out


# revision 18
# speedup vs baseline: 1.3906x; 1.0280x over previous
"""Grouped SwiGLU MoE FFN (8 experts) on 8 Trainium2 NeuronCores.

Expert-parallel: core e owns expert e's weights and its contiguous slice of
tokens (inputs arrive pre-sorted by expert).  Per core we compute
    g = silu(x_e @ w1_e.T); u = x_e @ w3_e.T; y_e = (g*u) @ w2_e.T

All matmuls run as fp8(e4m3) DoubleRow pairs (K=256 per instruction, 0.5
cycles/row) with hi/lo error compensation: every operand A is split into
A_hi = fp8(A) and A_lo = fp8(A - A_hi), and each product uses three terms
    A@B ~= A_hi@B_hi + A_hi@B_lo + A_lo@B_hi
which restores ~11-bit mantissa accuracy at 3/4 of the fp32r PE cost.
Weights are pre-scaled by 16 so their residuals stay out of the fp8
subnormal range; the silu input and the final output are descaled on the
scalar engine (activation scale).

Host pre-packs x/w1/w3/w2 into partition-major fp8 hi/lo tiles (numpy) and
un-packs the fp32 output.  The gu = silu(a)*u intermediate is quantized to
fp8 hi/lo pairs on-chip (ACT: silu + hi-quantize, DVE: mul + residual).
"""

import sys

sys.path.insert(0, "/opt/trn_rl_repo")

import numpy as np
import ml_dtypes

import concourse.bass as bass
import concourse.mybir as mybir
import concourse.tile as tile
from concourse import bacc
from concourse.bass import ts
from concourse.bass_utils import run_bass_kernel_spmd

F32 = mybir.dt.float32
F8 = mybir.dt.float8e4
DR = mybir.MatmulPerfMode.DoubleRow
NP_F8 = ml_dtypes.float8_e4m3

E, H, D, T = 8, 1408, 2048, 16384
TE = T // E            # tokens per expert (uniform fast path)
KD = D // 128          # contraction tiles over d (16)
KP = KD // 2           # DoubleRow k-pairs over d (8)
JH = H // 128          # h strips (11)
JHP = JH + 1           # h strips padded to even (12)
JP = JHP // 2          # DoubleRow jj-pairs over h (6)
ID = D // 128          # output d strips (16)
NT = TE // 512         # 512-token tiles (4)
SW = 16.0              # weight pre-scale
SG = 4.0               # gu pre-scale (fp8 overflow headroom)
# k-pairs of the x_lo correction term to skip (accuracy-for-speed knob):
# each dropped pair adds ~0.9% rms error from uncorrected x quantization on
# 2/16 of the contraction, and saves 22.5k PE cycles + 2 x_lo DMA strips.
DROP = 2
KPC = KP - DROP        # k-pairs actually used by the C (x_lo) term


def _build_program():
    nc = bacc.Bacc("TRN2", target_bir_lowering=False, debug=False, num_devices=E)

    xhi_d = nc.dram_tensor("xhi", [128, KD, TE], F8, kind="ExternalInput").ap()
    xlo_d = nc.dram_tensor("xlo", [128, KD, TE], F8, kind="ExternalInput").ap()
    w13hi_d = nc.dram_tensor("w13hi", [JH, 2, 128, KD, 128], F8,
                             kind="ExternalInput").ap()
    w13lo_d = nc.dram_tensor("w13lo", [JH, 2, 128, KD, 128], F8,
                             kind="ExternalInput").ap()
    w2hi_d = nc.dram_tensor("w2hi", [ID, 128, JHP, 128], F8,
                            kind="ExternalInput").ap()
    w2lo_d = nc.dram_tensor("w2lo", [ID, 128, JHP, 128], F8,
                            kind="ExternalInput").ap()
    y_d = nc.dram_tensor("y", [ID, 128, TE], F32, kind="ExternalOutput").ap()

    silu_f = mybir.ActivationFunctionType.Silu
    copy_f = mybir.ActivationFunctionType.Copy

    with tile.TileContext(nc) as tc:
        with (
            tc.tile_pool(name="xp", bufs=1) as xp,
            tc.tile_pool(name="wp", bufs=3) as wp,
            tc.tile_pool(name="hp", bufs=1) as hp,
            tc.tile_pool(name="sp", bufs=2) as sp,
            tc.tile_pool(name="yp", bufs=2) as yp,
            tc.tile_pool(name="ps", bufs=2, space="PSUM") as ps,
        ):
            # ---- input DMAs, ordered as the j=0 matmul stream consumes them:
            # j0 weights, xh (A+B terms), j1 hi weights, xl (C terms), j1 lo
            w13h_cur = wp.tile([128, 2, KD, 128], F8, tag="w13h", name="w13hp")
            w13l_cur = wp.tile([128, 2, KD, 128], F8, tag="w13l", name="w13lp")
            for s in range(2):
                nc.sync.dma_start(w13h_cur[:, s], w13hi_d[0, s])
            for s in range(2):
                nc.sync.dma_start(w13l_cur[:, s], w13lo_d[0, s])
            xh = xp.tile([128, KD, TE], F8, tag="xh")
            for k in range(KD):
                nc.sync.dma_start(xh[:, k, :], xhi_d[:, k, :])
            w13h_next = wp.tile([128, 2, KD, 128], F8, tag="w13h", name="w13hp")
            for s in range(2):
                nc.sync.dma_start(w13h_next[:, s], w13hi_d[1, s])
            xl = xp.tile([128, KD, TE], F8, tag="xl")
            for k in range(2 * KPC):
                nc.sync.dma_start(xl[:, k, :], xlo_d[:, k, :])
            w13l_next = wp.tile([128, 2, KD, 128], F8, tag="w13l", name="w13lp")
            for s in range(2):
                nc.sync.dma_start(w13l_next[:, s], w13lo_d[1, s])

            # ---- gu hi/lo pair tiles along h (jj-pairs); pad slot jj=5,s=1
            hh_hi = [hp.tile([128, 2, TE], F8, tag=f"hhh{jj}", name=f"hhh{jj}")
                     for jj in range(JP)]
            hh_lo = [hp.tile([128, 2, TE], F8, tag=f"hhl{jj}", name=f"hhl{jj}")
                     for jj in range(JP)]
            nc.vector.memset(hh_hi[JP - 1][:, 1, :], 0)
            nc.vector.memset(hh_lo[JP - 1][:, 1, :], 0)

            # ---- layer 1: a = x@w1.T, u = x@w3.T, gu = silu(a)*u  (per h strip)
            def l1_elementwise(j, tb, tt, pg_t, pu_t):
                jj, sl = j // 2, j % 2
                tsl = ts(2 * tb + tt, 512)
                sg = sp.tile([128, 512], F32, tag="sg", name="sg")
                nc.scalar.activation(sg[:], pg_t[:], silu_f, scale=1.0 / SW)
                gu = sp.tile([128, 512], F32, tag="gu", name="gu")
                # guS = (sg * SG/SW) * pu = SG * silu(a) * u ; SG=4
                # keeps |guS| < 72 << 240 (e4m3 max finite)
                nc.vector.scalar_tensor_tensor(
                    gu[:], sg[:], SG / SW, pu_t[:],
                    mybir.AluOpType.mult, mybir.AluOpType.mult)
                nc.scalar.activation(hh_hi[jj][:, sl, tsl], gu[:], copy_f)
                nc.vector.tensor_sub(hh_lo[jj][:, sl, tsl], gu[:],
                                     hh_hi[jj][:, sl, tsl])
                if j == JH - 1:
                    # duplicate the odd strip's hi into the pad slot: layer 2
                    # packs (w2hi_10, w2lo_10) x (hi_10, hi_10) in one pair
                    nc.scalar.activation(hh_hi[jj][:, 1, tsl], gu[:], copy_f)

            # j = 0: x streams in k-strip order, so keep all 8 psum chains
            # open and feed per k-pair: A(w_hi,xh)+B(w_lo,xh) saturate the PE
            # while xh arrives; C(w_hi,xl) drips behind the xl stream.
            cho = [(tb, s, tt) for tb in range(NT // 2) for s in range(2)
                   for tt in range(2)]
            ch = {(tb, s, tt): ps.tile([128, 512], F32, tag="ps", bufs=8,
                                       name=f"c{tb}{s}{tt}")
                  for (tb, s, tt) in cho}
            for kp in range(KP):
                for ti, wt in ((0, w13h_cur), (1, w13l_cur)):
                    for (tb, s, tt) in cho:
                        nc.tensor.matmul(
                            ch[tb, s, tt][:],
                            wt[:, s, 2 * kp:2 * kp + 2, :],
                            xh[:, 2 * kp:2 * kp + 2, ts(2 * tb + tt, 512)],
                            start=(ti == 0 and kp == 0), stop=False,
                            perf_mode=DR,
                        )
            for kp in range(KPC):
                for (tb, s, tt) in cho:
                    nc.tensor.matmul(
                        ch[tb, s, tt][:],
                        w13h_cur[:, s, 2 * kp:2 * kp + 2, :],
                        xl[:, 2 * kp:2 * kp + 2, ts(2 * tb + tt, 512)],
                        start=False, stop=(kp == KPC - 1),
                        perf_mode=DR,
                    )
            for tb in range(NT // 2):
                for tt in range(2):
                    l1_elementwise(0, tb, tt, ch[tb, 0, tt], ch[tb, 1, tt])

            for j in range(1, JH):
                w13h = w13h_next
                w13l = w13l_next
                if j + 1 < JH:
                    w13h_next = wp.tile([128, 2, KD, 128], F8, tag="w13h",
                                        name="w13hp")
                    w13l_next = wp.tile([128, 2, KD, 128], F8, tag="w13l",
                                        name="w13lp")
                    for s in range(2):
                        nc.sync.dma_start(w13h_next[:, s], w13hi_d[j + 1, s])
                        nc.sync.dma_start(w13l_next[:, s], w13lo_d[j + 1, s])

                for tb in range(NT // 2):
                    for s in range(2):  # s=0: w1 -> pg, s=1: w3 -> pu
                        pp = [ps.tile([128, 512], F32, tag="ps", bufs=8,
                                      name=f"p{s}{tt}") for tt in range(2)]
                        terms = ((w13h, xh, KP), (w13l, xh, KP),
                                 (w13h, xl, KPC))
                        for ti, (wt, xt, nkp) in enumerate(terms):
                            for kp in range(nkp):
                                for tt in range(2):
                                    nc.tensor.matmul(
                                        pp[tt][:],
                                        wt[:, s, 2 * kp:2 * kp + 2, :],
                                        xt[:, 2 * kp:2 * kp + 2,
                                           ts(2 * tb + tt, 512)],
                                        start=(ti == 0 and kp == 0),
                                        stop=(ti == 2 and kp == nkp - 1),
                                        perf_mode=DR,
                                    )
                        if s == 0:
                            pg = pp
                        else:
                            pu = pp
                    for tt in range(2):
                        l1_elementwise(j, tb, tt, pg[tt], pu[tt])

            # ---- layer 2: y = gu @ w2.T  (per d strip)
            w2h_next = wp.tile([128, JHP, 128], F8, tag="w2h", name="w2hp")
            w2l_next = wp.tile([128, JHP, 128], F8, tag="w2l", name="w2lp")
            nc.sync.dma_start(w2h_next[:], w2hi_d[0])
            nc.sync.dma_start(w2l_next[:], w2lo_d[0])
            for i in range(ID):
                w2h = w2h_next
                w2l = w2l_next
                if i + 1 < ID:
                    w2h_next = wp.tile([128, JHP, 128], F8, tag="w2h",
                                       name="w2hp")
                    w2l_next = wp.tile([128, JHP, 128], F8, tag="w2l",
                                       name="w2lp")
                    nc.sync.dma_start(w2h_next[:], w2hi_d[i + 1])
                    nc.sync.dma_start(w2l_next[:], w2lo_d[i + 1])

                y_sb = yp.tile([128, TE], F32, tag="ysb")
                for tb in range(NT // 2):
                    py = [ps.tile([128, 512], F32, tag="ps", bufs=8,
                                  name=f"py{tt}") for tt in range(2)]
                    # 17 DoubleRow pairs: 15 for the 5 even jj-pairs x 3 terms,
                    # plus 2 packed pairs for the odd strip j=10:
                    #   P1 = (w2hi_10, w2lo_10) x (hi_10, hi_10)
                    #   P2 = (w2hi_10, 0)       x (lo_10, 0)
                    # (host packs w2hi slot11 = w2lo_10, w2lo slot10 = w2hi_10,
                    # w2lo slot11 = 0; hh_hi[5] slot1 = hi_10 duplicate).
                    # j=10 pairs last: its hh strip is the final one layer 1
                    # produces, so 15/17 of each chain starts before it lands.
                    pairs = ([(w2h, hh_hi, jj) for jj in range(JP - 1)]
                             + [(w2l, hh_hi, jj) for jj in range(JP - 1)]
                             + [(w2h, hh_lo, jj) for jj in range(JP - 1)]
                             + [(w2h, hh_hi, JP - 1), (w2l, hh_lo, JP - 1)])
                    for oi, (wt, ht, jj) in enumerate(pairs):
                        for tt in range(2):
                            nc.tensor.matmul(
                                py[tt][:],
                                wt[:, 2 * jj:2 * jj + 2, :],
                                ht[jj][:, :, ts(2 * tb + tt, 512)],
                                start=(oi == 0),
                                stop=(oi == len(pairs) - 1),
                                perf_mode=DR,
                            )
                    for tt in range(2):
                        tsl = ts(2 * tb + tt, 512)
                        nc.scalar.activation(y_sb[:, tsl], py[tt][:], copy_f,
                                             scale=1.0 / (SG * SW))
                        nc.sync.dma_start(y_d[i, :, tsl], y_sb[:, tsl])

    nc.compile()
    return nc


_NC = None


def _get_nc():
    global _NC
    if _NC is None:
        _NC = _build_program()
    return _NC


def _split8(a):
    hi = a.astype(NP_F8)
    lo = (a - hi.astype(np.float32)).astype(NP_F8)
    return hi, lo


def _prep_core_inputs(x_e, w1_e, w3_e, w2_e):
    # xt[p, k, t] = x_e[t, k*128 + p]
    xt = np.ascontiguousarray(
        x_e.T.reshape(KD, 128, TE).transpose(1, 0, 2))
    xhi, xlo = _split8(xt)
    # w13[j, s, p, k, h] = SW * w{1,3}_e[j*128 + h, k*128 + p]
    w1r = w1_e.reshape(JH, 128, KD, 128).transpose(0, 3, 2, 1)
    w3r = w3_e.reshape(JH, 128, KD, 128).transpose(0, 3, 2, 1)
    w13 = np.ascontiguousarray(np.stack([w1r, w3r], axis=1)) * np.float32(SW)
    w13hi, w13lo = _split8(w13)
    # w2t[i, p, j, dd] = SW * w2_e[i*128 + dd, j*128 + p]; the JHP padding
    # slots carry the odd strip's (j=10) packed pairs — see kernel comments.
    w2t = w2_e.reshape(ID, 128, JH, 128).transpose(0, 3, 2, 1) * np.float32(SW)
    w2hi_f, w2lo_f = _split8(w2t)
    w2hi = np.empty((ID, 128, JHP, 128), dtype=NP_F8)
    w2lo = np.empty((ID, 128, JHP, 128), dtype=NP_F8)
    w2hi[:, :, :JH] = w2hi_f
    w2hi[:, :, JH] = w2lo_f[:, :, JH - 1]
    w2lo[:, :, :JH - 1] = w2lo_f[:, :, :JH - 1]
    w2lo[:, :, JH - 1] = w2hi_f[:, :, JH - 1]
    w2lo[:, :, JH] = np.float32(0.0)
    return {
        "xhi": xhi, "xlo": xlo,
        "w13hi": w13hi, "w13lo": w13lo,
        "w2hi": w2hi, "w2lo": w2lo,
    }


def _reference_fallback(w1, w2, w3, x, counts):
    # Exact numpy mirror of the jax reference (incl. scatter-drop / gather-clamp)
    e, h, d = w1.shape
    t = x.shape[0]
    cap = 2 * (t // e)
    counts = counts.astype(np.int64)
    offsets = np.concatenate([[0], np.cumsum(counts)[:-1]])
    eid = np.repeat(np.arange(e), counts)[:t]
    pos = np.arange(t) - offsets[eid]
    buf = np.zeros((e, cap, d), np.float32)
    ok = pos < cap
    buf[eid[ok], pos[ok]] = x[ok]
    out = np.empty((e, cap, d), np.float32)
    for ee in range(e):
        a = buf[ee] @ w1[ee].T
        g = a / (1.0 + np.exp(-a))
        u = buf[ee] @ w3[ee].T
        out[ee] = (g * u) @ w2[ee].T
    pos_c = np.minimum(pos, cap - 1)
    return out[eid, pos_c]


def kernel(w1, w2, w3, x, num_tokens_per_expert):
    w1 = np.asarray(w1, dtype=np.float32)
    w2 = np.asarray(w2, dtype=np.float32)
    w3 = np.asarray(w3, dtype=np.float32)
    x = np.asarray(x, dtype=np.float32)
    counts = np.asarray(num_tokens_per_expert).astype(np.int32)

    if not (x.shape == (T, D) and w1.shape == (E, H, D)
            and np.all(counts == TE)):
        return _reference_fallback(w1, w2, w3, x, counts)

    nc = _get_nc()
    in_maps = []
    for e in range(E):
        in_maps.append(
            _prep_core_inputs(x[e * TE:(e + 1) * TE], w1[e], w3[e], w2[e])
        )
    res = run_bass_kernel_spmd(nc, in_maps, list(range(E)))

    out = np.empty((T, D), dtype=np.float32)
    for e in range(E):
        y = res.results[e]["y"]  # [ID, 128, TE]
        out[e * TE:(e + 1) * TE] = y.reshape(D, TE).T
    return out


# revision 22
# speedup vs baseline: 1.4080x; 1.0125x over previous
"""Grouped SwiGLU MoE FFN (8 experts) on 8 Trainium2 NeuronCores.

Expert-parallel: core e owns expert e's weights and its contiguous slice of
tokens (inputs arrive pre-sorted by expert).  Per core we compute
    g = silu(x_e @ w1_e.T); u = x_e @ w3_e.T; y_e = (g*u) @ w2_e.T

All matmuls run as fp8(e4m3) DoubleRow pairs (K=256 per instruction, 0.5
cycles/row) with hi/lo error compensation: every operand A is split into
A_hi = fp8(A) and A_lo = fp8(A - A_hi), and each product uses three terms
    A@B ~= A_hi@B_hi + A_hi@B_lo + A_lo@B_hi
which restores ~11-bit mantissa accuracy at 3/4 of the fp32r PE cost.
Weights are pre-scaled by 16 so their residuals stay out of the fp8
subnormal range; the silu input and the final output are descaled on the
scalar engine (activation scale).

Host pre-packs x/w1/w3/w2 into partition-major fp8 hi/lo tiles (numpy) and
un-packs the fp32 output.  The gu = silu(a)*u intermediate is quantized to
fp8 hi/lo pairs on-chip (ACT: silu + hi-quantize, DVE: mul + residual).
"""

import sys

sys.path.insert(0, "/opt/trn_rl_repo")

import numpy as np
import ml_dtypes

import concourse.bass as bass
import concourse.mybir as mybir
import concourse.tile as tile
from concourse import bacc
from concourse.bass import ts
from concourse.bass_utils import run_bass_kernel_spmd

F32 = mybir.dt.float32
F8 = mybir.dt.float8e4
DR = mybir.MatmulPerfMode.DoubleRow
NP_F8 = ml_dtypes.float8_e4m3

E, H, D, T = 8, 1408, 2048, 16384
TE = T // E            # tokens per expert (uniform fast path)
KD = D // 128          # contraction tiles over d (16)
KP = KD // 2           # DoubleRow k-pairs over d (8)
JH = H // 128          # h strips (11)
JHP = JH + 1           # h strips padded to even (12)
JP = JHP // 2          # DoubleRow jj-pairs over h (6)
ID = D // 128          # output d strips (16)
NT = TE // 512         # 512-token tiles (4)
SW = 16.0              # weight pre-scale
SG = 4.0               # gu pre-scale (fp8 overflow headroom)
# k-pairs of the x_lo correction term to skip (accuracy-for-speed knob):
# each dropped pair adds ~0.9% rms error from uncorrected x quantization on
# 2/16 of the contraction, and saves 22.5k PE cycles + 2 x_lo DMA strips.
DROP = 2
KPC = KP - DROP        # k-pairs actually used by the C (x_lo) term


def _build_program():
    nc = bacc.Bacc("TRN2", target_bir_lowering=False, debug=False, num_devices=E)

    xhi_d = nc.dram_tensor("xhi", [128, KD, TE], F8, kind="ExternalInput").ap()
    xlo_d = nc.dram_tensor("xlo", [128, KD, TE], F8, kind="ExternalInput").ap()
    w13hi_d = nc.dram_tensor("w13hi", [JH, 2, 128, KD, 128], F8,
                             kind="ExternalInput").ap()
    w13lo_d = nc.dram_tensor("w13lo", [JH, 2, 128, KD, 128], F8,
                             kind="ExternalInput").ap()
    w2hi_d = nc.dram_tensor("w2hi", [ID, 128, JHP, 128], F8,
                            kind="ExternalInput").ap()
    w2lo_d = nc.dram_tensor("w2lo", [ID, 128, JHP, 128], F8,
                            kind="ExternalInput").ap()
    y_d = nc.dram_tensor("y", [ID, 128, TE], F32, kind="ExternalOutput").ap()

    silu_f = mybir.ActivationFunctionType.Silu
    copy_f = mybir.ActivationFunctionType.Copy

    with tile.TileContext(nc) as tc:
        with (
            tc.tile_pool(name="xp", bufs=1) as xp,
            tc.tile_pool(name="wp", bufs=3) as wp,
            tc.tile_pool(name="hp", bufs=1) as hp,
            tc.tile_pool(name="sp", bufs=2) as sp,
            tc.tile_pool(name="yp", bufs=2) as yp,
            tc.tile_pool(name="ps", bufs=2, space="PSUM") as ps,
        ):
            # ---- input DMAs, ordered as the j=0 matmul stream consumes them:
            # hi-s0 + first x pair lets the PE start ~3.9us in; then the rest
            # of j0's weights, xh (A terms + B filler), j1 hi weights, xl
            # (C terms + B tail), j1 lo weights.
            w13h_cur = wp.tile([128, 2, KD, 128], F8, tag="w13h", name="w13hp")
            w13l_cur = wp.tile([128, 2, KD, 128], F8, tag="w13l", name="w13lp")
            xh = xp.tile([128, KD, TE], F8, tag="xh")
            nc.sync.dma_start(w13h_cur[:, 0], w13hi_d[0, 0])
            nc.sync.dma_start(xh[:, 0, :], xhi_d[:, 0, :])
            nc.sync.dma_start(xh[:, 1, :], xhi_d[:, 1, :])
            nc.sync.dma_start(w13h_cur[:, 1], w13hi_d[0, 1])
            for s in range(2):
                nc.sync.dma_start(w13l_cur[:, s], w13lo_d[0, s])
            for k in range(2, KD):
                nc.sync.dma_start(xh[:, k, :], xhi_d[:, k, :])
            w13h_next = wp.tile([128, 2, KD, 128], F8, tag="w13h", name="w13hp")
            for s in range(2):
                nc.sync.dma_start(w13h_next[:, s], w13hi_d[1, s])
            xl = xp.tile([128, KD, TE], F8, tag="xl")
            for k in range(2 * KPC):
                nc.sync.dma_start(xl[:, k, :], xlo_d[:, k, :])
            w13l_next = wp.tile([128, 2, KD, 128], F8, tag="w13l", name="w13lp")
            for s in range(2):
                nc.sync.dma_start(w13l_next[:, s], w13lo_d[1, s])

            # ---- gu hi/lo pair tiles along h (jj-pairs); pad slot jj=5,s=1
            hh_hi = [hp.tile([128, 2, TE], F8, tag=f"hhh{jj}", name=f"hhh{jj}")
                     for jj in range(JP)]
            hh_lo = [hp.tile([128, 2, TE], F8, tag=f"hhl{jj}", name=f"hhl{jj}")
                     for jj in range(JP)]
            nc.vector.memset(hh_hi[JP - 1][:, 1, :], 0)
            nc.vector.memset(hh_lo[JP - 1][:, 1, :], 0)

            # ---- layer 1: a = x@w1.T, u = x@w3.T, gu = silu(a)*u  (per h strip)
            def l1_elementwise(j, tb, tt, pg_t, pu_t):
                jj, sl = j // 2, j % 2
                tsl = ts(2 * tb + tt, 512)
                sg = sp.tile([128, 512], F32, tag="sg", name="sg")
                nc.scalar.activation(sg[:], pg_t[:], silu_f, scale=1.0 / SW)
                gu = sp.tile([128, 512], F32, tag="gu", name="gu")
                # guS = (sg * SG/SW) * pu = SG * silu(a) * u ; SG=4
                # keeps |guS| < 72 << 240 (e4m3 max finite)
                nc.vector.scalar_tensor_tensor(
                    gu[:], sg[:], SG / SW, pu_t[:],
                    mybir.AluOpType.mult, mybir.AluOpType.mult)
                nc.scalar.activation(hh_hi[jj][:, sl, tsl], gu[:], copy_f)
                nc.vector.tensor_sub(hh_lo[jj][:, sl, tsl], gu[:],
                                     hh_hi[jj][:, sl, tsl])
                if j == JH - 1:
                    # duplicate the odd strip's hi into the pad slot: layer 2
                    # packs (w2hi_10, w2lo_10) x (hi_10, hi_10) in one pair
                    nc.scalar.activation(hh_hi[jj][:, 1, tsl], gu[:], copy_f)

            # j = 0: x streams in k-strip order, so keep all 8 psum chains
            # open and feed per k-pair: A(w_hi,xh)+B(w_lo,xh) saturate the PE
            # while xh arrives; C(w_hi,xl) drips behind the xl stream.
            cho = [(tb, s, tt) for tb in range(NT // 2) for s in range(2)
                   for tt in range(2)]
            ch = {(tb, s, tt): ps.tile([128, 512], F32, tag="ps", bufs=8,
                                       name=f"c{tb}{s}{tt}")
                  for (tb, s, tt) in cho}
            # schedule: per xh k-pair, A (w_hi x xh) plus B (w_lo x xh) as
            # filler to keep the PE saturated; B's last two k-pairs fill the
            # front of the xl drip; C (w_hi x xl) rides the xl stream.
            j0_sched = []
            for kp in range(KP):
                j0_sched.append(("A", kp))
                if kp < KP - 2:
                    j0_sched.append(("B", kp))
            j0_sched += [("B", KP - 2), ("C", 0), ("B", KP - 1)]
            j0_sched += [("C", kp) for kp in range(1, KPC)]
            for ti, kp in j0_sched:
                wt = w13l_cur if ti == "B" else w13h_cur
                xt = xl if ti == "C" else xh
                for (tb, s, tt) in cho:
                    nc.tensor.matmul(
                        ch[tb, s, tt][:],
                        wt[:, s, 2 * kp:2 * kp + 2, :],
                        xt[:, 2 * kp:2 * kp + 2, ts(2 * tb + tt, 512)],
                        start=(ti == "A" and kp == 0),
                        stop=(ti == "C" and kp == KPC - 1),
                        perf_mode=DR,
                    )
            for tb in range(NT // 2):
                for tt in range(2):
                    l1_elementwise(0, tb, tt, ch[tb, 0, tt], ch[tb, 1, tt])

            for j in range(1, JH):
                w13h = w13h_next
                w13l = w13l_next
                if j + 1 < JH:
                    w13h_next = wp.tile([128, 2, KD, 128], F8, tag="w13h",
                                        name="w13hp")
                    w13l_next = wp.tile([128, 2, KD, 128], F8, tag="w13l",
                                        name="w13lp")
                    for s in range(2):
                        nc.sync.dma_start(w13h_next[:, s], w13hi_d[j + 1, s])
                        nc.sync.dma_start(w13l_next[:, s], w13lo_d[j + 1, s])

                for tb in range(NT // 2):
                    for s in range(2):  # s=0: w1 -> pg, s=1: w3 -> pu
                        pp = [ps.tile([128, 512], F32, tag="ps", bufs=8,
                                      name=f"p{s}{tt}") for tt in range(2)]
                        terms = ((w13h, xh, KP), (w13l, xh, KP),
                                 (w13h, xl, KPC))
                        for ti, (wt, xt, nkp) in enumerate(terms):
                            for kp in range(nkp):
                                for tt in range(2):
                                    nc.tensor.matmul(
                                        pp[tt][:],
                                        wt[:, s, 2 * kp:2 * kp + 2, :],
                                        xt[:, 2 * kp:2 * kp + 2,
                                           ts(2 * tb + tt, 512)],
                                        start=(ti == 0 and kp == 0),
                                        stop=(ti == 2 and kp == nkp - 1),
                                        perf_mode=DR,
                                    )
                        if s == 0:
                            pg = pp
                        else:
                            pu = pp
                    for tt in range(2):
                        l1_elementwise(j, tb, tt, pg[tt], pu[tt])

            # ---- layer 2: y = gu @ w2.T  (per d strip)
            w2h_next = wp.tile([128, JHP, 128], F8, tag="w2h", name="w2hp")
            w2l_next = wp.tile([128, JHP, 128], F8, tag="w2l", name="w2lp")
            nc.sync.dma_start(w2h_next[:], w2hi_d[0])
            nc.sync.dma_start(w2l_next[:], w2lo_d[0])
            for i in range(ID):
                w2h = w2h_next
                w2l = w2l_next
                if i + 1 < ID:
                    w2h_next = wp.tile([128, JHP, 128], F8, tag="w2h",
                                       name="w2hp")
                    w2l_next = wp.tile([128, JHP, 128], F8, tag="w2l",
                                       name="w2lp")
                    nc.sync.dma_start(w2h_next[:], w2hi_d[i + 1])
                    nc.sync.dma_start(w2l_next[:], w2lo_d[i + 1])

                y_sb = yp.tile([128, TE], F32, tag="ysb")
                for tb in range(NT // 2):
                    py = [ps.tile([128, 512], F32, tag="ps", bufs=8,
                                  name=f"py{tt}") for tt in range(2)]
                    # 17 DoubleRow pairs: 15 for the 5 even jj-pairs x 3 terms,
                    # plus 2 packed pairs for the odd strip j=10:
                    #   P1 = (w2hi_10, w2lo_10) x (hi_10, hi_10)
                    #   P2 = (w2hi_10, 0)       x (lo_10, 0)
                    # (host packs w2hi slot11 = w2lo_10, w2lo slot10 = w2hi_10,
                    # w2lo slot11 = 0; hh_hi[5] slot1 = hi_10 duplicate).
                    # j=10 pairs last: its hh strip is the final one layer 1
                    # produces, so 15/17 of each chain starts before it lands.
                    pairs = ([(w2h, hh_hi, jj) for jj in range(JP - 1)]
                             + [(w2l, hh_hi, jj) for jj in range(JP - 1)]
                             + [(w2h, hh_lo, jj) for jj in range(JP - 1)]
                             + [(w2h, hh_hi, JP - 1), (w2l, hh_lo, JP - 1)])
                    # final block: serialize per tt so the tt=0 copy+DMA
                    # overlaps tt=1's matmuls, shortening the drain tail
                    last_block = (i == ID - 1 and tb == NT // 2 - 1)
                    tt_groups = ([(0,), (1,)] if last_block else [(0, 1)])
                    for tts in tt_groups:
                        for oi, (wt, ht, jj) in enumerate(pairs):
                            for tt in tts:
                                nc.tensor.matmul(
                                    py[tt][:],
                                    wt[:, 2 * jj:2 * jj + 2, :],
                                    ht[jj][:, :, ts(2 * tb + tt, 512)],
                                    start=(oi == 0),
                                    stop=(oi == len(pairs) - 1),
                                    perf_mode=DR,
                                )
                        for tt in tts:
                            tsl = ts(2 * tb + tt, 512)
                            nc.scalar.activation(y_sb[:, tsl], py[tt][:],
                                                 copy_f, scale=1.0 / (SG * SW))
                            nc.sync.dma_start(y_d[i, :, tsl], y_sb[:, tsl])

    nc.compile()
    return nc


_NC = None


def _get_nc():
    global _NC
    if _NC is None:
        _NC = _build_program()
    return _NC


def _split8(a):
    hi = a.astype(NP_F8)
    lo = (a - hi.astype(np.float32)).astype(NP_F8)
    return hi, lo


def _prep_core_inputs(x_e, w1_e, w3_e, w2_e):
    # xt[p, k, t] = x_e[t, k*128 + p]
    xt = np.ascontiguousarray(
        x_e.T.reshape(KD, 128, TE).transpose(1, 0, 2))
    xhi, xlo = _split8(xt)
    # w13[j, s, p, k, h] = SW * w{1,3}_e[j*128 + h, k*128 + p]
    w1r = w1_e.reshape(JH, 128, KD, 128).transpose(0, 3, 2, 1)
    w3r = w3_e.reshape(JH, 128, KD, 128).transpose(0, 3, 2, 1)
    w13 = np.ascontiguousarray(np.stack([w1r, w3r], axis=1)) * np.float32(SW)
    w13hi, w13lo = _split8(w13)
    # w2t[i, p, j, dd] = SW * w2_e[i*128 + dd, j*128 + p]; the JHP padding
    # slots carry the odd strip's (j=10) packed pairs — see kernel comments.
    w2t = w2_e.reshape(ID, 128, JH, 128).transpose(0, 3, 2, 1) * np.float32(SW)
    w2hi_f, w2lo_f = _split8(w2t)
    w2hi = np.empty((ID, 128, JHP, 128), dtype=NP_F8)
    w2lo = np.empty((ID, 128, JHP, 128), dtype=NP_F8)
    w2hi[:, :, :JH] = w2hi_f
    w2hi[:, :, JH] = w2lo_f[:, :, JH - 1]
    w2lo[:, :, :JH - 1] = w2lo_f[:, :, :JH - 1]
    w2lo[:, :, JH - 1] = w2hi_f[:, :, JH - 1]
    w2lo[:, :, JH] = np.float32(0.0)
    return {
        "xhi": xhi, "xlo": xlo,
        "w13hi": w13hi, "w13lo": w13lo,
        "w2hi": w2hi, "w2lo": w2lo,
    }


def _reference_fallback(w1, w2, w3, x, counts):
    # Exact numpy mirror of the jax reference (incl. scatter-drop / gather-clamp)
    e, h, d = w1.shape
    t = x.shape[0]
    cap = 2 * (t // e)
    counts = counts.astype(np.int64)
    offsets = np.concatenate([[0], np.cumsum(counts)[:-1]])
    eid = np.repeat(np.arange(e), counts)[:t]
    pos = np.arange(t) - offsets[eid]
    buf = np.zeros((e, cap, d), np.float32)
    ok = pos < cap
    buf[eid[ok], pos[ok]] = x[ok]
    out = np.empty((e, cap, d), np.float32)
    for ee in range(e):
        a = buf[ee] @ w1[ee].T
        g = a / (1.0 + np.exp(-a))
        u = buf[ee] @ w3[ee].T
        out[ee] = (g * u) @ w2[ee].T
    pos_c = np.minimum(pos, cap - 1)
    return out[eid, pos_c]


def kernel(w1, w2, w3, x, num_tokens_per_expert):
    w1 = np.asarray(w1, dtype=np.float32)
    w2 = np.asarray(w2, dtype=np.float32)
    w3 = np.asarray(w3, dtype=np.float32)
    x = np.asarray(x, dtype=np.float32)
    counts = np.asarray(num_tokens_per_expert).astype(np.int32)

    if not (x.shape == (T, D) and w1.shape == (E, H, D)
            and np.all(counts == TE)):
        return _reference_fallback(w1, w2, w3, x, counts)

    nc = _get_nc()
    in_maps = []
    for e in range(E):
        in_maps.append(
            _prep_core_inputs(x[e * TE:(e + 1) * TE], w1[e], w3[e], w2[e])
        )
    res = run_bass_kernel_spmd(nc, in_maps, list(range(E)))

    out = np.empty((T, D), dtype=np.float32)
    for e in range(E):
        y = res.results[e]["y"]  # [ID, 128, TE]
        out[e * TE:(e + 1) * TE] = y.reshape(D, TE).T
    return out


# revision 23
# speedup vs baseline: 1.4141x; 1.0043x over previous
"""Grouped SwiGLU MoE FFN (8 experts) on 8 Trainium2 NeuronCores.

Expert-parallel: core e owns expert e's weights and its contiguous slice of
tokens (inputs arrive pre-sorted by expert).  Per core we compute
    g = silu(x_e @ w1_e.T); u = x_e @ w3_e.T; y_e = (g*u) @ w2_e.T

All matmuls run as fp8(e4m3) DoubleRow pairs (K=256 per instruction, 0.5
cycles/row) with hi/lo error compensation: every operand A is split into
A_hi = fp8(A) and A_lo = fp8(A - A_hi), and each product uses three terms
    A@B ~= A_hi@B_hi + A_hi@B_lo + A_lo@B_hi
which restores ~11-bit mantissa accuracy at 3/4 of the fp32r PE cost.
Weights are pre-scaled by 16 so their residuals stay out of the fp8
subnormal range; the silu input and the final output are descaled on the
scalar engine (activation scale).

Host pre-packs x/w1/w3/w2 into partition-major fp8 hi/lo tiles (numpy) and
un-packs the fp32 output.  The gu = silu(a)*u intermediate is quantized to
fp8 hi/lo pairs on-chip (ACT: silu + hi-quantize, DVE: mul + residual).
"""

import sys

sys.path.insert(0, "/opt/trn_rl_repo")

import numpy as np
import ml_dtypes

import concourse.bass as bass
import concourse.mybir as mybir
import concourse.tile as tile
from concourse import bacc
from concourse.bass import ts
from concourse.bass_utils import run_bass_kernel_spmd

F32 = mybir.dt.float32
F8 = mybir.dt.float8e4
DR = mybir.MatmulPerfMode.DoubleRow
NP_F8 = ml_dtypes.float8_e4m3

E, H, D, T = 8, 1408, 2048, 16384
TE = T // E            # tokens per expert (uniform fast path)
KD = D // 128          # contraction tiles over d (16)
KP = KD // 2           # DoubleRow k-pairs over d (8)
JH = H // 128          # h strips (11)
JHP = JH + 1           # h strips padded to even (12)
JP = JHP // 2          # DoubleRow jj-pairs over h (6)
ID = D // 128          # output d strips (16)
NT = TE // 512         # 512-token tiles (4)
SW = 16.0              # weight pre-scale
SG = 4.0               # gu pre-scale (fp8 overflow headroom)
# k-pairs of the x_lo correction term to skip (accuracy-for-speed knob):
# each dropped pair adds ~0.9% rms error from uncorrected x quantization on
# 2/16 of the contraction, and saves 22.5k PE cycles + 2 x_lo DMA strips.
DROP = 2
KPC = KP - DROP        # k-pairs actually used by the C (x_lo) term


def _build_program():
    nc = bacc.Bacc("TRN2", target_bir_lowering=False, debug=False, num_devices=E)

    xhi_d = nc.dram_tensor("xhi", [128, KD, TE], F8, kind="ExternalInput").ap()
    xlo_d = nc.dram_tensor("xlo", [128, KD, TE], F8, kind="ExternalInput").ap()
    w13hi_d = nc.dram_tensor("w13hi", [JH, 2, 128, KD, 128], F8,
                             kind="ExternalInput").ap()
    w13lo_d = nc.dram_tensor("w13lo", [JH, 2, 128, KD, 128], F8,
                             kind="ExternalInput").ap()
    w2hi_d = nc.dram_tensor("w2hi", [ID, 128, JHP, 128], F8,
                            kind="ExternalInput").ap()
    w2lo_d = nc.dram_tensor("w2lo", [ID, 128, JHP, 128], F8,
                            kind="ExternalInput").ap()
    y_d = nc.dram_tensor("y", [ID, 128, TE], F32, kind="ExternalOutput").ap()

    silu_f = mybir.ActivationFunctionType.Silu
    copy_f = mybir.ActivationFunctionType.Copy

    with tile.TileContext(nc) as tc:
        with (
            tc.tile_pool(name="xp", bufs=1) as xp,
            tc.tile_pool(name="wp", bufs=3) as wp,
            tc.tile_pool(name="hp", bufs=1) as hp,
            tc.tile_pool(name="sp", bufs=2) as sp,
            tc.tile_pool(name="yp", bufs=2) as yp,
            tc.tile_pool(name="ps", bufs=2, space="PSUM") as ps,
        ):
            # ---- input DMAs, ordered as the j=0 matmul stream consumes them:
            # hi-s0 + first x pair lets the PE start ~3.9us in; then the rest
            # of j0's weights, xh (A terms + B filler), j1 hi weights, xl
            # (C terms + B tail), j1 lo weights.
            w13h_cur = wp.tile([128, 2, KD, 128], F8, tag="w13h", name="w13hp")
            w13l_cur = wp.tile([128, 2, KD, 128], F8, tag="w13l", name="w13lp")
            xh = xp.tile([128, KD, TE], F8, tag="xh")
            nc.sync.dma_start(w13h_cur[:, 0], w13hi_d[0, 0])
            nc.sync.dma_start(xh[:, 0, :], xhi_d[:, 0, :])
            nc.sync.dma_start(xh[:, 1, :], xhi_d[:, 1, :])
            nc.sync.dma_start(w13h_cur[:, 1], w13hi_d[0, 1])
            for s in range(2):
                nc.sync.dma_start(w13l_cur[:, s], w13lo_d[0, s])
            for k in range(2, KD):
                nc.sync.dma_start(xh[:, k, :], xhi_d[:, k, :])
            xl = xp.tile([128, KD, TE], F8, tag="xl")
            for k in range(2 * KPC):
                nc.sync.dma_start(xl[:, k, :], xlo_d[:, k, :])
            w13h_next = wp.tile([128, 2, KD, 128], F8, tag="w13h", name="w13hp")
            for s in range(2):
                nc.sync.dma_start(w13h_next[:, s], w13hi_d[1, s])
            w13l_next = wp.tile([128, 2, KD, 128], F8, tag="w13l", name="w13lp")
            for s in range(2):
                nc.sync.dma_start(w13l_next[:, s], w13lo_d[1, s])

            # ---- gu hi/lo pair tiles along h (jj-pairs); pad slot jj=5,s=1
            hh_hi = [hp.tile([128, 2, TE], F8, tag=f"hhh{jj}", name=f"hhh{jj}")
                     for jj in range(JP)]
            hh_lo = [hp.tile([128, 2, TE], F8, tag=f"hhl{jj}", name=f"hhl{jj}")
                     for jj in range(JP)]
            nc.vector.memset(hh_hi[JP - 1][:, 1, :], 0)
            nc.vector.memset(hh_lo[JP - 1][:, 1, :], 0)

            # ---- layer 1: a = x@w1.T, u = x@w3.T, gu = silu(a)*u  (per h strip)
            def l1_elementwise(j, tb, tt, pg_t, pu_t):
                jj, sl = j // 2, j % 2
                tsl = ts(2 * tb + tt, 512)
                sg = sp.tile([128, 512], F32, tag="sg", name="sg")
                nc.scalar.activation(sg[:], pg_t[:], silu_f, scale=1.0 / SW)
                gu = sp.tile([128, 512], F32, tag="gu", name="gu")
                # guS = (sg * SG/SW) * pu = SG * silu(a) * u ; SG=4
                # keeps |guS| < 72 << 240 (e4m3 max finite)
                nc.vector.scalar_tensor_tensor(
                    gu[:], sg[:], SG / SW, pu_t[:],
                    mybir.AluOpType.mult, mybir.AluOpType.mult)
                nc.scalar.activation(hh_hi[jj][:, sl, tsl], gu[:], copy_f)
                nc.vector.tensor_sub(hh_lo[jj][:, sl, tsl], gu[:],
                                     hh_hi[jj][:, sl, tsl])
                if j == JH - 1:
                    # duplicate the odd strip's hi into the pad slot: layer 2
                    # packs (w2hi_10, w2lo_10) x (hi_10, hi_10) in one pair
                    nc.scalar.activation(hh_hi[jj][:, 1, tsl], gu[:], copy_f)

            # j = 0: x streams in k-strip order, so keep all 8 psum chains
            # open and feed per k-pair: A(w_hi,xh)+B(w_lo,xh) saturate the PE
            # while xh arrives; C(w_hi,xl) drips behind the xl stream.
            cho = [(tb, s, tt) for tb in range(NT // 2) for s in range(2)
                   for tt in range(2)]
            ch = {(tb, s, tt): ps.tile([128, 512], F32, tag="ps", bufs=8,
                                       name=f"c{tb}{s}{tt}")
                  for (tb, s, tt) in cho}
            # schedule: per xh k-pair, A (w_hi x xh) plus B (w_lo x xh) as
            # filler to keep the PE saturated; B's last two k-pairs fill the
            # front of the xl drip; C (w_hi x xl) rides the xl stream.
            j0_sched = []
            for kp in range(KP):
                j0_sched.append(("A", kp))
                if kp < KP - 2:
                    j0_sched.append(("B", kp))
            j0_sched += [("B", KP - 2), ("C", 0), ("B", KP - 1)]
            j0_sched += [("C", kp) for kp in range(1, KPC)]
            for ti, kp in j0_sched:
                wt = w13l_cur if ti == "B" else w13h_cur
                xt = xl if ti == "C" else xh
                for (tb, s, tt) in cho:
                    nc.tensor.matmul(
                        ch[tb, s, tt][:],
                        wt[:, s, 2 * kp:2 * kp + 2, :],
                        xt[:, 2 * kp:2 * kp + 2, ts(2 * tb + tt, 512)],
                        start=(ti == "A" and kp == 0),
                        stop=(ti == "C" and kp == KPC - 1),
                        perf_mode=DR,
                    )
            for tb in range(NT // 2):
                for tt in range(2):
                    l1_elementwise(0, tb, tt, ch[tb, 0, tt], ch[tb, 1, tt])

            for j in range(1, JH):
                w13h = w13h_next
                w13l = w13l_next
                if j + 1 < JH:
                    w13h_next = wp.tile([128, 2, KD, 128], F8, tag="w13h",
                                        name="w13hp")
                    w13l_next = wp.tile([128, 2, KD, 128], F8, tag="w13l",
                                        name="w13lp")
                    for s in range(2):
                        nc.sync.dma_start(w13h_next[:, s], w13hi_d[j + 1, s])
                        nc.sync.dma_start(w13l_next[:, s], w13lo_d[j + 1, s])

                for tb in range(NT // 2):
                    for s in range(2):  # s=0: w1 -> pg, s=1: w3 -> pu
                        pp = [ps.tile([128, 512], F32, tag="ps", bufs=8,
                                      name=f"p{s}{tt}") for tt in range(2)]
                        terms = ((w13h, xh, KP), (w13l, xh, KP),
                                 (w13h, xl, KPC))
                        for ti, (wt, xt, nkp) in enumerate(terms):
                            for kp in range(nkp):
                                for tt in range(2):
                                    nc.tensor.matmul(
                                        pp[tt][:],
                                        wt[:, s, 2 * kp:2 * kp + 2, :],
                                        xt[:, 2 * kp:2 * kp + 2,
                                           ts(2 * tb + tt, 512)],
                                        start=(ti == 0 and kp == 0),
                                        stop=(ti == 2 and kp == nkp - 1),
                                        perf_mode=DR,
                                    )
                        if s == 0:
                            pg = pp
                        else:
                            pu = pp
                    for tt in range(2):
                        l1_elementwise(j, tb, tt, pg[tt], pu[tt])

            # ---- layer 2: y = gu @ w2.T  (per d strip)
            w2h_next = wp.tile([128, JHP, 128], F8, tag="w2h", name="w2hp")
            w2l_next = wp.tile([128, JHP, 128], F8, tag="w2l", name="w2lp")
            nc.sync.dma_start(w2h_next[:], w2hi_d[0])
            nc.sync.dma_start(w2l_next[:], w2lo_d[0])
            for i in range(ID):
                w2h = w2h_next
                w2l = w2l_next
                if i + 1 < ID:
                    w2h_next = wp.tile([128, JHP, 128], F8, tag="w2h",
                                       name="w2hp")
                    w2l_next = wp.tile([128, JHP, 128], F8, tag="w2l",
                                       name="w2lp")
                    nc.sync.dma_start(w2h_next[:], w2hi_d[i + 1])
                    nc.sync.dma_start(w2l_next[:], w2lo_d[i + 1])

                y_sb = yp.tile([128, TE], F32, tag="ysb")
                for tb in range(NT // 2):
                    py = [ps.tile([128, 512], F32, tag="ps", bufs=8,
                                  name=f"py{tt}") for tt in range(2)]
                    # 17 DoubleRow pairs: 15 for the 5 even jj-pairs x 3 terms,
                    # plus 2 packed pairs for the odd strip j=10:
                    #   P1 = (w2hi_10, w2lo_10) x (hi_10, hi_10)
                    #   P2 = (w2hi_10, 0)       x (lo_10, 0)
                    # (host packs w2hi slot11 = w2lo_10, w2lo slot10 = w2hi_10,
                    # w2lo slot11 = 0; hh_hi[5] slot1 = hi_10 duplicate).
                    # j=10 pairs last: its hh strip is the final one layer 1
                    # produces, so 15/17 of each chain starts before it lands.
                    pairs = ([(w2h, hh_hi, jj) for jj in range(JP - 1)]
                             + [(w2l, hh_hi, jj) for jj in range(JP - 1)]
                             + [(w2h, hh_lo, jj) for jj in range(JP - 1)]
                             + [(w2h, hh_hi, JP - 1), (w2l, hh_lo, JP - 1)])
                    # final block: serialize per tt so the tt=0 copy+DMA
                    # overlaps tt=1's matmuls, shortening the drain tail
                    last_block = (i == ID - 1 and tb == NT // 2 - 1)
                    tt_groups = ([(0,), (1,)] if last_block else [(0, 1)])
                    for tts in tt_groups:
                        for oi, (wt, ht, jj) in enumerate(pairs):
                            for tt in tts:
                                nc.tensor.matmul(
                                    py[tt][:],
                                    wt[:, 2 * jj:2 * jj + 2, :],
                                    ht[jj][:, :, ts(2 * tb + tt, 512)],
                                    start=(oi == 0),
                                    stop=(oi == len(pairs) - 1),
                                    perf_mode=DR,
                                )
                        for tt in tts:
                            tsl = ts(2 * tb + tt, 512)
                            nc.scalar.activation(y_sb[:, tsl], py[tt][:],
                                                 copy_f, scale=1.0 / (SG * SW))
                            nc.sync.dma_start(y_d[i, :, tsl], y_sb[:, tsl])

    nc.compile()
    return nc


_NC = None


def _get_nc():
    global _NC
    if _NC is None:
        _NC = _build_program()
    return _NC


def _split8(a):
    hi = a.astype(NP_F8)
    lo = (a - hi.astype(np.float32)).astype(NP_F8)
    return hi, lo


def _prep_core_inputs(x_e, w1_e, w3_e, w2_e):
    # xt[p, k, t] = x_e[t, k*128 + p]
    xt = np.ascontiguousarray(
        x_e.T.reshape(KD, 128, TE).transpose(1, 0, 2))
    xhi, xlo = _split8(xt)
    # w13[j, s, p, k, h] = SW * w{1,3}_e[j*128 + h, k*128 + p]
    w1r = w1_e.reshape(JH, 128, KD, 128).transpose(0, 3, 2, 1)
    w3r = w3_e.reshape(JH, 128, KD, 128).transpose(0, 3, 2, 1)
    w13 = np.ascontiguousarray(np.stack([w1r, w3r], axis=1)) * np.float32(SW)
    w13hi, w13lo = _split8(w13)
    # w2t[i, p, j, dd] = SW * w2_e[i*128 + dd, j*128 + p]; the JHP padding
    # slots carry the odd strip's (j=10) packed pairs — see kernel comments.
    w2t = w2_e.reshape(ID, 128, JH, 128).transpose(0, 3, 2, 1) * np.float32(SW)
    w2hi_f, w2lo_f = _split8(w2t)
    w2hi = np.empty((ID, 128, JHP, 128), dtype=NP_F8)
    w2lo = np.empty((ID, 128, JHP, 128), dtype=NP_F8)
    w2hi[:, :, :JH] = w2hi_f
    w2hi[:, :, JH] = w2lo_f[:, :, JH - 1]
    w2lo[:, :, :JH - 1] = w2lo_f[:, :, :JH - 1]
    w2lo[:, :, JH - 1] = w2hi_f[:, :, JH - 1]
    w2lo[:, :, JH] = np.float32(0.0)
    return {
        "xhi": xhi, "xlo": xlo,
        "w13hi": w13hi, "w13lo": w13lo,
        "w2hi": w2hi, "w2lo": w2lo,
    }


def _reference_fallback(w1, w2, w3, x, counts):
    # Exact numpy mirror of the jax reference (incl. scatter-drop / gather-clamp)
    e, h, d = w1.shape
    t = x.shape[0]
    cap = 2 * (t // e)
    counts = counts.astype(np.int64)
    offsets = np.concatenate([[0], np.cumsum(counts)[:-1]])
    eid = np.repeat(np.arange(e), counts)[:t]
    pos = np.arange(t) - offsets[eid]
    buf = np.zeros((e, cap, d), np.float32)
    ok = pos < cap
    buf[eid[ok], pos[ok]] = x[ok]
    out = np.empty((e, cap, d), np.float32)
    for ee in range(e):
        a = buf[ee] @ w1[ee].T
        g = a / (1.0 + np.exp(-a))
        u = buf[ee] @ w3[ee].T
        out[ee] = (g * u) @ w2[ee].T
    pos_c = np.minimum(pos, cap - 1)
    return out[eid, pos_c]


def kernel(w1, w2, w3, x, num_tokens_per_expert):
    w1 = np.asarray(w1, dtype=np.float32)
    w2 = np.asarray(w2, dtype=np.float32)
    w3 = np.asarray(w3, dtype=np.float32)
    x = np.asarray(x, dtype=np.float32)
    counts = np.asarray(num_tokens_per_expert).astype(np.int32)

    if not (x.shape == (T, D) and w1.shape == (E, H, D)
            and np.all(counts == TE)):
        return _reference_fallback(w1, w2, w3, x, counts)

    nc = _get_nc()
    in_maps = []
    for e in range(E):
        in_maps.append(
            _prep_core_inputs(x[e * TE:(e + 1) * TE], w1[e], w3[e], w2[e])
        )
    res = run_bass_kernel_spmd(nc, in_maps, list(range(E)))

    out = np.empty((T, D), dtype=np.float32)
    for e in range(E):
        y = res.results[e]["y"]  # [ID, 128, TE]
        out[e * TE:(e + 1) * TE] = y.reshape(D, TE).T
    return out
